# revision 29
# baseline (speedup 1.0000x reference)
"""AngularDescriptor Trainium2 kernel (8 NeuronCores, SPMD + AllReduce).

Per core: T/8 triplets.  Device computes Chebyshev/Legendre bases, the
per-pair-type radial einsum (PE matmul with fixed block-diag weights after a
4-way tj/tk one-hot expansion; 4-way ti select on DVE), the outer product
ang = (g_ij*g_ik) (x) P_l, and segment-sums ang into q[20000,8,4] via
gpsimd.dma_scatter_add.  HW scatter-add loses duplicate indices within one
instruction (last-write-wins race), so the host orders each shard's triplets
into occurrence-rank classes (class r = r-th occurrence of an atom): within a
class all atom indices are unique.  Classes are cut into chunks that rotate
over K DRAM accumulators, so same-accumulator scatters serialize (WAW dep)
while different-accumulator scatters overlap.  Padding slots scatter to
distinct dummy atom rows (20000..20479) so one uniform program serves all
cores.  Final: on-device K-way add, AllReduce over the 8 cores, output from
core 0.

v2 layout notes:
 - Chebyshev T_k computed on the Activation engine: with s=(r/rc-1)^2,
   theta = arccos(2s-1) = 2*atan(sqrt(1/s-1)), T_k = sin(pi/2 - 2k*atan(.)).
   The "+1" of the reference basis is folded into the weight table.
 - F1 features are feature-major [p, half, q, k, j] and built with one fused
   DVE op per half; PE transposes merge 2 j-columns per instruction, with the
   weight matrix rows permuted to the (feat,jparity)-interleaved order.
 - A block's SEG=4 triplets occupy 4 consecutive columns of one partition, so
   the per-block pre-reduction is 3 DVE adds (no PE matmuls).
"""
import sys

sys.path.insert(0, "/opt/trn_rl_repo")
import numpy as np

from concourse import bass, bacc, mybir, tile
from concourse.bass_utils import run_bass_kernel_spmd

N_TYPES, N_DESC, K_MAX, L_MAX = 4, 8, 8, 4
R_C = 5.0
N_ATOMS = 20000
N_CORES = 8
DL = N_DESC * L_MAX          # 32
QPAD = 20480                 # 128 * 160
STRIDE = 64                  # q row stride in f32 (256B; scatter needs 256B mult)
J = 128                      # field columns per macro-tile
MACRO = 128 * J              # 16384 triplets per macro
KACC = 2                     # rotating DRAM accumulators
CHUNK = 4096                 # max idxs (blocks) per scatter instruction
SEG = 4                      # triplets pre-reduced per scattered block
F32, BF16, I16 = mybir.dt.float32, mybir.dt.bfloat16, mybir.dt.int16
PI = float(np.pi)
CHEB_ATAN = False            # activation-engine Chebyshev (HW Sin/Arctan input
                             # ranges are too narrow; use the DVE recurrence)


def _host_prep(n_atoms, triplet_index, r_ij, r_ik, cos_theta,
               type_i, type_j, type_k, c_table):
    """Shard; group each atom's triplets into SEG-slot blocks (pad slots use
    r=r_c so fc=0 => ang=0); order blocks by block-rank classes; pad to a
    uniform layout across cores.  Block b's slots live at partition b%128,
    columns SEG*(b//128)..+SEG-1 (block-major columns)."""
    T = triplet_index.shape[0]
    Tc = T // N_CORES
    atom_all = np.asarray(triplet_index[:, 0], dtype=np.int64)

    per_core = []
    max_nblk = 0
    for c in range(N_CORES):
        sl = slice(c * Tc, (c + 1) * Tc)
        atom = atom_all[sl]
        order = np.argsort(atom, kind="stable")
        sa = atom[order]
        first = np.r_[True, sa[1:] != sa[:-1]]
        idxf = np.where(first)[0]
        counts = np.diff(np.r_[idxf, Tc])
        uatoms = sa[idxf]
        nblk_per_atom = -(-counts // SEG)
        per_core.append((sl, atom, order, uatoms, counts, nblk_per_atom))
        max_nblk = max(max_nblk, int(nblk_per_atom.max()))

    nclass = max_nblk
    # per class r: number of blocks (padded to 128) -- uniform across cores
    cls_blk = []
    for r in range(nclass):
        mx = max(int(np.sum(nb > r)) for (_, _, _, _, _, nb) in per_core)
        cls_blk.append(-(-max(mx, 1) // 128) * 128)
    total_blk = sum(cls_blk)
    MB = MACRO // SEG                     # blocks per macro
    nmacro = -(-total_blk // MB)
    TBLK = nmacro * MB
    TPAD = TBLK * SEG

    # chunk table in blocks: (start_blk, len_blk); aligned to macros
    chunks = []
    o = 0
    for r in range(nclass):
        p = cls_blk[r]
        sblk = 0
        while sblk < p:
            cl = min(CHUNK, p - sblk)
            mstart = (o + sblk) // MB
            if (o + sblk + cl - 1) // MB != mstart:
                cl = (mstart + 1) * MB - (o + sblk)
            chunks.append((o + sblk, cl))
            sblk += cl
        o += p
    tail = o
    while tail < TBLK:
        cl = min(CHUNK, TBLK - tail, ((tail // MB) + 1) * MB - tail)
        chunks.append((tail, cl))
        tail += cl

    G = TPAD // 128
    cores = []
    for c in range(N_CORES):
        sl, atom, order, uatoms, counts, nblk = per_core[c]
        f32 = lambda x: np.asarray(x[sl], dtype=np.float32)
        fields = dict(r_ij=f32(r_ij), r_ik=f32(r_ik), ct=f32(cos_theta),
                      ti=f32(type_i), tj=f32(type_j), tk=f32(type_k))
        # pad slots: r=R_C -> fc=0 -> ang=0 exactly
        dev = {}
        for n in fields:
            fillv = R_C if n in ("r_ij", "r_ik") else 0.0
            dev[n] = np.full((128, G), fillv, dtype=np.float32)
        bidx = np.empty(TBLK, dtype=np.int16)
        bidx[:] = (20000 + (np.arange(TBLK, dtype=np.int64) % 480)).astype(np.int16)

        # block table: for each unique atom, blocks 0..nblk-1; block b holds
        # triplets order[start + b*SEG : ...] (clipped); class-major placement
        starts = np.r_[0, np.cumsum(counts)[:-1]]
        o = 0
        for r in range(nclass):
            members = np.where(nblk > r)[0]         # atoms having block r
            if len(members):
                a_st = starts[members] + r * SEG
                a_ct = np.minimum(counts[members] - r * SEG, SEG)
                blk = o + np.arange(len(members))   # destination block ids
                src_idx = []
                dst_p = []
                dst_c = []
                for st, ctn, b in zip(a_st, a_ct, blk):
                    src_idx.append(order[st:st + ctn])
                    dst_p.append(np.full(ctn, b % 128))
                    dst_c.append(SEG * (b // 128) + np.arange(ctn))
                src_idx = np.concatenate(src_idx)
                dst_p = np.concatenate(dst_p)
                dst_c = np.concatenate(dst_c)
                for n in dev:
                    dev[n][dst_p, dst_c] = fields[n][src_idx]
                bidx[o:o + len(members)] = uatoms[members].astype(np.int16)
            o += cls_blk[r]
        arrays = {n: dev[n] for n in dev}
        # idx wrapped [16, TBLK/16] and replicated over the 8 gpsimd cores
        arrays["idx"] = np.tile(bidx.reshape(TBLK // 16, 16).T, (8, 1)).copy()
        cores.append(arrays)

    # weight table with the (T_k + 1) -> T_k fold:
    #   sum_k c[d,k]*(T_k+1)*u = sum_k c'[d,k]*T_k*u,  c'[d,0] += sum_k c[d,k]
    ctab = np.asarray(c_table, dtype=np.float64).copy()
    ctab[:, :, :, 0] += ctab.sum(axis=3)
    ctab = ctab.astype(np.float32)
    W = np.zeros((32, 32), dtype=np.float32)
    for tjv in range(4):
        for k in range(8):
            W[tjv * 8 + k, :] = ctab[:, tjv, :, k].reshape(32)
    W4 = np.zeros((128, 128), dtype=np.float32)
    for b in range(4):
        W4[b * 32:(b + 1) * 32, b * 32:(b + 1) * 32] = W
    # permute rows to the (feat, jparity)-interleaved transpose output order:
    # new row (2f+jj) = old row (jj*64+f)
    W4p = np.zeros_like(W4)
    for jj in range(2):
        for f in range(64):
            W4p[2 * f + jj, :] = W4[jj * 64 + f, :]
    consts = dict(w4=W4p, ident=np.eye(128, dtype=np.float32),
                  iota4=np.tile(np.arange(4, dtype=np.float32), (128, 1)))
    return cores, consts, chunks, nmacro, TPAD


def _build(chunks, nmacro, TPAD, debug=False):
    G = TPAD // 128
    MB = MACRO // SEG
    nc = bacc.Bacc(None, target_bir_lowering=False, num_devices=N_CORES,
                   dynamic_dma_scratch_size=32768, num_swdge_queues=2)
    dbg = {}
    if debug:
        dbg["f1"] = nc.declare_dram_parameter("dbg_f1", [128, 64 * J], F32,
                                              isOutput=True)
        dbg["prodc"] = nc.declare_dram_parameter("dbg_prodc", [128, J * 32], F32,
                                                 isOutput=True)
        dbg["partials"] = nc.declare_dram_parameter("dbg_partials", [128, 32 * 32],
                                                    F32, isOutput=True)
    P = {}
    for n in ("r_ij", "r_ik", "ct", "ti", "tj", "tk"):
        P[n] = nc.declare_dram_parameter(n, [128, G], F32, isOutput=False)
    P["idx"] = nc.declare_dram_parameter("idx", [128, TPAD // SEG // 16], I16,
                                         isOutput=False)
    P["w4"] = nc.declare_dram_parameter("w4", [128, 128], F32, isOutput=False)
    P["ident"] = nc.declare_dram_parameter("ident", [128, 128], F32, isOutput=False)
    P["iota4"] = nc.declare_dram_parameter("iota4", [128, 4], F32, isOutput=False)
    out_d = nc.declare_dram_parameter("out", [N_ATOMS, DL], F32, isOutput=True)

    qacc = [nc.dram_tensor(f"qacc{k}", [QPAD, STRIDE], F32) for k in range(KACC)]
    bounce_in = nc.dram_tensor("bounce_in", [128, QPAD * DL // 128], F32)
    bounce_out = nc.dram_tensor("bounce_out", [128, QPAD * DL // 128], F32,
                                addr_space="Shared")

    AF = mybir.ActivationFunctionType
    OP = mybir.AluOpType

    with tile.TileContext(nc) as tc:
        with tc.tile_pool(name="const", bufs=1) as cst:
            w4 = cst.tile([128, 128], BF16)
            ident = cst.tile([128, 128], BF16)
            iota4 = cst.tile([128, 4], F32)
            tmpf = cst.tile([128, 128], F32)
            zero = cst.tile([128, 2048], F32)
            halfpi = cst.tile([128, 1], F32)
            nc.vector.memset(halfpi[:], PI / 2)
            negone = cst.tile([128, 1], F32)
            nc.vector.memset(negone[:], -1.0)
            dbg_t = cst.tile([128, 1024], F32, name="dbg_t") if debug else None
            nc.sync.dma_start(out=tmpf[:], in_=P["w4"][:])
            nc.vector.tensor_copy(out=w4[:], in_=tmpf[:])
            nc.sync.dma_start(out=tmpf[:], in_=P["ident"][:])
            nc.vector.tensor_copy(out=ident[:], in_=tmpf[:])
            nc.sync.dma_start(out=iota4[:], in_=P["iota4"][:])
            nc.vector.memset(zero[:], 0.0)
            qf0 = qacc[0].ap().rearrange("(p r) s -> p (r s)", p=128)
            w0 = QPAD * STRIDE // 128
            for i in range(0, w0, 2048):
                nc.sync.dma_start(out=qf0[:, i:i + 2048],
                                  in_=zero[:, :min(2048, w0 - i)])

            with (
                tc.tile_pool(name="fields", bufs=3) as fpool,
                tc.tile_pool(name="idxp", bufs=3) as ipool,
                tc.tile_pool(name="work", bufs=2) as wpool,
                tc.tile_pool(name="cinp", bufs=4) as cpool,
                tc.tile_pool(name="angp", bufs=2) as apool,
                tc.tile_pool(name="scat", bufs=3) as spool,
                tc.tile_pool(name="ps1", bufs=3, space="PSUM") as ppool1,
                tc.tile_pool(name="ps2", bufs=3, space="PSUM") as ppool2,
            ):
                by_macro = [[] for _ in range(nmacro)]
                for ci, (s, pl) in enumerate(chunks):
                    by_macro[s // MB].append((ci, s % MB, pl))

                for m in range(nmacro):
                    fld = {}
                    for n in ("r_ij", "r_ik", "ct", "ti", "tj", "tk"):
                        t = fpool.tile([128, J], F32)
                        nc.sync.dma_start(out=t[:], in_=P[n][:, m * J:(m + 1) * J])
                        fld[n] = t
                    NIC = MB // 16   # idx cols per macro
                    idxs = ipool.tile([128, NIC], I16)
                    nc.sync.dma_start(out=idxs[:],
                                      in_=P["idx"][:, m * NIC:(m + 1) * NIC])
                    if m == 0:
                        # qacc1 zeroing rides behind macro-0 loads; its first
                        # scatter is a macro later (qacc0 was zeroed upfront)
                        for k in range(1, KACC):
                            qf = qacc[k].ap().rearrange("(p r) s -> p (r s)", p=128)
                            w = QPAD * STRIDE // 128
                            for i in range(0, w, 2048):
                                nc.sync.dma_start(out=qf[:, i:i + 2048],
                                                  in_=zero[:, :min(2048, w - i)])

                    # ---- bases + one-hot expanded features ----
                    # layout [p, jpair, feat=(h,q,k), jparity]: a j-pair's 128
                    # features are contiguous -> one full-width PE transpose
                    F1 = wpool.tile([128, J // 2, 64, 2], BF16)
                    for half, (rn, tn) in enumerate((("r_ij", "tj"), ("r_ik", "tk"))):
                        r = fld[rn]
                        u = wpool.tile([128, J], F32)
                        nc.scalar.activation(u[:], r[:], AF.Sin,
                                             bias=halfpi[:], scale=-PI / R_C)
                        nc.vector.tensor_scalar(out=u[:], in0=u[:], scalar1=0.25,
                                                scalar2=0.25, op0=OP.mult, op1=OP.add)
                        ub = wpool.tile([128, J], BF16)
                        nc.vector.tensor_copy(out=ub[:], in_=u[:])
                        s = wpool.tile([128, J], F32)
                        nc.scalar.activation(s[:], r[:], AF.Square,
                                             bias=negone[:], scale=1.0 / R_C)
                        x = wpool.tile([128, J], F32)
                        nc.vector.tensor_scalar(out=x[:], in0=s[:], scalar1=2.0,
                                                scalar2=-1.0, op0=OP.mult,
                                                op1=OP.add)
                        x2 = wpool.tile([128, J], F32)
                        nc.vector.tensor_scalar_mul(x2[:], x[:], 2.0)
                        Traw = wpool.tile([128, 8, J], F32)
                        nc.vector.memset(Traw[:, 0, :], 1.0)
                        nc.vector.tensor_copy(out=Traw[:, 1, :], in_=x[:])
                        for k in range(2, 8):
                            nc.vector.tensor_tensor(out=Traw[:, k, :], in0=x2[:],
                                                    in1=Traw[:, k - 1, :],
                                                    op=OP.mult)
                            nc.vector.tensor_tensor(out=Traw[:, k, :],
                                                    in0=Traw[:, k, :],
                                                    in1=Traw[:, k - 2, :],
                                                    op=OP.subtract)
                        # Tu[k] = T_k * u
                        Tu = wpool.tile([128, 8, J], BF16)
                        nc.vector.tensor_tensor(
                            out=Tu[:], in0=Traw[:],
                            in1=ub[:].unsqueeze(1).broadcast_to([128, 8, J]),
                            op=OP.mult)
                        # oh[q] = (tq == q)
                        oh = wpool.tile([128, 4, J], BF16)
                        tq = fld[tn]
                        nc.vector.tensor_tensor(
                            out=oh[:],
                            in0=tq[:].unsqueeze(1).broadcast_to([128, 4, J]),
                            in1=iota4[:].unsqueeze(2).broadcast_to([128, 4, J]),
                            op=OP.is_equal)
                        # F1[jp, (half,q,k), jj] = Tu[k, 2jp+jj] * oh[q, 2jp+jj]
                        Tus = Tu[:].rearrange("p k (jp jj) -> p k jp jj", jj=2)
                        ohs = oh[:].rearrange("p q (jp jj) -> p q jp jj", jj=2)
                        Fv = F1[:].rearrange("p jp (h q k) jj -> p h q k jp jj",
                                             h=2, q=4)
                        for jj in range(2):
                            nc.vector.tensor_tensor(
                                out=Fv[:, half, :, :, :, jj],
                                in0=Tus[:, :, :, jj].unsqueeze(1)
                                    .broadcast_to([128, 4, 8, J // 2]),
                                in1=ohs[:, :, :, jj].unsqueeze(2)
                                    .broadcast_to([128, 4, 8, J // 2]),
                                op=OP.mult)

                    oh_ti = wpool.tile([128, J, 4], BF16)
                    nc.vector.tensor_tensor(
                        out=oh_ti[:],
                        in0=fld["ti"][:].unsqueeze(2).broadcast_to([128, J, 4]),
                        in1=iota4[:].unsqueeze(1).broadcast_to([128, J, 4]),
                        op=OP.is_equal)

                    # Legendre (bf16): P0=1, P1=ct, P2=1.5ct^2-.5, P3=ct(2.5ct^2-1.5)
                    ctf = fld["ct"]
                    P4 = wpool.tile([128, J, 4], BF16)
                    ct2 = wpool.tile([128, J], F32)
                    nc.vector.tensor_tensor(out=ct2[:], in0=ctf[:], in1=ctf[:],
                                            op=OP.mult)
                    nc.vector.memset(P4[:, :, 0], 1.0)
                    nc.vector.tensor_copy(out=P4[:, :, 1], in_=ctf[:])
                    nc.vector.tensor_scalar(out=P4[:, :, 2], in0=ct2[:], scalar1=1.5,
                                            scalar2=-0.5, op0=OP.mult, op1=OP.add)
                    p3t = wpool.tile([128, J], F32)
                    nc.vector.tensor_scalar(out=p3t[:], in0=ct2[:], scalar1=2.5,
                                            scalar2=-1.5, op0=OP.mult, op1=OP.add)
                    nc.vector.tensor_tensor(out=p3t[:], in0=p3t[:], in1=ctf[:],
                                            op=OP.mult)
                    nc.vector.tensor_copy(out=P4[:, :, 3], in_=p3t[:])

                    # ---- PE: per 8-column group ----
                    prodc = wpool.tile([128, J, 32], BF16)
                    F1m = F1[:].rearrange("p jp f jj -> p jp (f jj)")
                    NG = J // 8
                    for grp in range(NG):
                        j0 = grp * 8
                        ps1 = ppool1.tile([128, 4, 128], BF16, space="PSUM")
                        for c in range(4):
                            nc.tensor.transpose(
                                out=ps1[:, c, :], in_=F1m[:, 4 * grp + c, :],
                                identity=ident[:])
                        cin = cpool.tile([128, 4, 128], BF16)
                        nc.scalar.activation(
                            cin[:].rearrange("p c f -> p (c f)"),
                            ps1[:].rearrange("p c f -> p (c f)"), AF.Identity)
                        ps2 = ppool2.tile([128, 4, 128], F32, space="PSUM")
                        for c in range(4):
                            nc.tensor.matmul(out=ps2[:, c, :], lhsT=cin[:, c, :],
                                             rhs=w4[:], start=True, stop=True)
                        cnd = cpool.tile([128, 4, 128], BF16)
                        nc.scalar.activation(
                            cnd[:].rearrange("p c f -> p (c f)"),
                            ps2[:].rearrange("p c f -> p (c f)"), AF.Identity)
                        # prodc[j0+2c+jj, f] = g_ij * g_ik
                        psv = cnd[:].rearrange("p c (j h f) -> p c j h f", j=2, h=2)
                        nc.vector.tensor_tensor(
                            out=prodc[:, j0:j0 + 8, :]
                                .rearrange("p (c j) f -> p c j f", c=4),
                            in0=psv[:, :, :, 0, :], in1=psv[:, :, :, 1, :],
                            op=OP.mult)

                    # ---- ti select: h = sum_q ohti[q] * prodc[:, :, q*8:(q+1)*8]
                    h = wpool.tile([128, J, 8], BF16)
                    t0 = wpool.tile([128, J, 8], BF16)
                    nc.vector.tensor_tensor(
                        out=h[:], in0=prodc[:, :, 0:8],
                        in1=oh_ti[:, :, 0].unsqueeze(2).broadcast_to([128, J, 8]),
                        op=OP.mult)
                    for q in range(1, 4):
                        nc.vector.tensor_tensor(
                            out=t0[:], in0=prodc[:, :, q * 8:(q + 1) * 8],
                            in1=oh_ti[:, :, q].unsqueeze(2).broadcast_to([128, J, 8]),
                            op=OP.mult)
                        nc.vector.tensor_tensor(out=h[:], in0=h[:], in1=t0[:],
                                                op=OP.add)

                    # ---- ang = h (x) P (bf16) ----
                    ang = apool.tile([128, J, 8, 4], BF16)
                    nc.vector.tensor_tensor(
                        out=ang[:],
                        in0=h[:].unsqueeze(3).broadcast_to([128, J, 8, 4]),
                        in1=P4[:].unsqueeze(2).broadcast_to([128, J, 8, 4]),
                        op=OP.mult)

                    # ---- per-block pre-reduction: 3 adds over 4 columns ----
                    partials = spool.tile([128, J // SEG, 32], F32)
                    av = ang[:].rearrange("p (g s) d l -> p g s (d l)", s=SEG)
                    ts1 = wpool.tile([128, J // SEG, 32], BF16)
                    ts2 = wpool.tile([128, J // SEG, 32], BF16)
                    nc.vector.tensor_tensor(out=ts1[:], in0=av[:, :, 0, :],
                                            in1=av[:, :, 1, :], op=OP.add)
                    nc.vector.tensor_tensor(out=ts2[:], in0=av[:, :, 2, :],
                                            in1=av[:, :, 3, :], op=OP.add)
                    nc.vector.tensor_tensor(out=partials[:], in0=ts1[:],
                                            in1=ts2[:], op=OP.add)

                    if debug and m == 0:
                        for nm, src, w in (
                            ("f1", F1[:].rearrange("p jp f jj -> p (jp f jj)"), 64 * J),
                            ("prodc", prodc[:].rearrange("p j f -> p (j f)"), J * 32),
                            ("partials", partials[:].rearrange("p g f -> p (g f)"),
                             32 * 32),
                        ):
                            for i in range(0, w, 1024):
                                e = min(i + 1024, w)
                                nc.vector.tensor_copy(out=dbg_t[:, :e - i],
                                                      in_=src[:, i:e])
                                nc.sync.dma_start(out=dbg[nm].ap()[:, i:e],
                                                  in_=dbg_t[:, :e - i])

                    # ---- scatter chunks of this macro (block granularity) ----
                    for (ci, off, pl) in by_macro[m]:
                        nc.gpsimd.dma_scatter_add(
                            qacc[ci % KACC].ap()[:, :DL],
                            partials[:, off // 128:(off + pl) // 128, :],
                            idxs[:, off // 16:(off + pl) // 16],
                            pl, pl, DL, elem_step=STRIDE,
                            queue_num=ci % 2)

            # ---- reduce K accumulators, AllReduce, emit ----
            with tc.tile_pool(name="red", bufs=1) as rpool:
                W = QPAD * STRIDE // 128   # 10240
                acc = rpool.tile([128, W], F32)
                tmp = rpool.tile([128, W], F32)
                qv = [q.ap().rearrange("(p r) s -> p (r s)", p=128) for q in qacc]
                nc.sync.dma_start(out=acc[:], in_=qv[0])
                nc.sync.dma_start(out=tmp[:], in_=qv[1])
                nc.vector.tensor_tensor(out=acc[:], in0=acc[:], in1=tmp[:], op=OP.add)
                packed = rpool.tile([128, QPAD * DL // 128], F32)  # [128, 5120]
                nc.vector.tensor_copy(
                    out=packed[:].rearrange("p (r s) -> p r s", s=DL),
                    in_=acc[:].rearrange("p (r s) -> p r s", s=STRIDE)[:, :, :DL])
                nc.sync.dma_start(out=bounce_in.ap(), in_=packed[:])
                nc.gpsimd.collective_compute(
                    "AllReduce", OP.add,
                    replica_groups=[list(range(N_CORES))],
                    ins=[bounce_in.ap().opt()], outs=[bounce_out.ap().opt()])
                nc.sync.dma_start(
                    out=out_d[:, :],
                    in_=bounce_out.ap().rearrange("p f -> (p f)")[
                        :N_ATOMS * DL].rearrange("(a c) -> a c", c=DL))
    nc.compile()
    return nc


def _install_ntff_hook():
    """Provide antenv.axon_hooks (missing in this image) via sys.modules so
    run_bass_kernel_spmd(trace=True) can capture NTFF profiles."""
    import types, ctypes, contextlib
    try:
        from antenv.axon_hooks import get_axon_ntff_profile_hook  # noqa: F401
        return
    except ImportError:
        pass
    so_path = "/opt/axon/libaxon_pjrt.so"
    try:
        lib = ctypes.CDLL(so_path)
    except OSError:
        return
    if not hasattr(lib, "axon_start_nrt_profile"):
        return
    lib.axon_start_nrt_profile.argtypes = [ctypes.POINTER(ctypes.c_int64),
                                           ctypes.c_size_t]
    lib.axon_start_nrt_profile.restype = ctypes.c_int64
    lib.axon_stop_nrt_profile.argtypes = [ctypes.c_char_p]
    lib.axon_stop_nrt_profile.restype = ctypes.c_int64

    @contextlib.contextmanager
    def _hook(output_dir, device_ids):
        import jax
        jax.devices()
        if device_ids:
            ids = (ctypes.c_int64 * len(device_ids))(*device_ids)
            rc = lib.axon_start_nrt_profile(ids, len(device_ids))
        else:
            rc = lib.axon_start_nrt_profile(None, 0)
        if rc != 0:
            raise RuntimeError(f"axon_start_nrt_profile rc={rc}")
        try:
            yield
        finally:
            n = lib.axon_stop_nrt_profile(str(output_dir).encode())
            if n <= 0:
                print(f"ntff capture wrote {n} files", flush=True)

    mod = types.ModuleType("antenv.axon_hooks")
    mod.get_axon_ntff_profile_hook = lambda: _hook
    mod.set_axon_ntff_profile_hook = lambda h: None
    import antenv
    sys.modules["antenv.axon_hooks"] = mod
    antenv.axon_hooks = mod



_CACHE = {}


def kernel(n_atoms, triplet_index, r_ij, r_ik, cos_theta,
           type_i, type_j, type_k, c_table, _sim=False, _trace=False):
    cores, consts, chunks, nmacro, TPAD = _host_prep(
        n_atoms, triplet_index, r_ij, r_ik, cos_theta,
        type_i, type_j, type_k, c_table)
    key = (nmacro, TPAD, tuple(chunks))
    if key not in _CACHE:
        _CACHE[key] = _build(chunks, nmacro, TPAD)
    nc = _CACHE[key]
    in_maps = []
    for c in range(N_CORES):
        m = dict(cores[c])
        m.update(consts)
        in_maps.append(m)
    if _sim:
        from concourse import bass_interp
        sim = bass_interp.MultiCoreSim(nc, N_CORES)
        for c in range(N_CORES):
            for k, v in in_maps[c].items():
                sim.cores[c].tensor(k)[:] = v
        sim.simulate()
        out = np.array(sim.cores[0].mem_tensor("out"))
    else:
        if _trace:
            _install_ntff_hook()
        last_err = None
        for _try in range(3):
            try:
                res = run_bass_kernel_spmd(nc, in_maps,
                                           core_ids=list(range(N_CORES)),
                                           trace=_trace)
                out = np.asarray(res.results[0]["out"])
                break
            except Exception as e:  # transient device-unrecoverable after a crash
                last_err = e
        else:
            raise last_err
        kernel.last_exec_ns = res.exec_time_ns
        kernel.last_results = res
    return out.reshape(N_ATOMS, N_DESC, L_MAX).astype(np.float32)


# revision 33
# speedup vs baseline: 1.2858x; 1.2858x over previous
"""AngularDescriptor Trainium2 kernel (8 NeuronCores, SPMD + AllReduce).

Per core: T/8 triplets.  Device computes Chebyshev/Legendre bases, the
per-pair-type radial einsum (PE matmul with fixed block-diag weights after a
4-way tj/tk one-hot expansion; 4-way ti select on DVE), the outer product
ang = (g_ij*g_ik) (x) P_l, and segment-sums ang into q[20000,8,4] via
gpsimd.dma_scatter_add.  HW scatter-add loses duplicate indices within one
instruction (last-write-wins race), so the host orders each shard's triplets
into occurrence-rank classes (class r = r-th occurrence of an atom): within a
class all atom indices are unique.  Classes are cut into chunks that rotate
over K DRAM accumulators, so same-accumulator scatters serialize (WAW dep)
while different-accumulator scatters overlap.  Padding slots scatter to
distinct dummy atom rows (20000..20479) so one uniform program serves all
cores.  Final: on-device K-way add, AllReduce over the 8 cores, output from
core 0.

v2 layout notes:
 - Chebyshev T_k computed on the Activation engine: with s=(r/rc-1)^2,
   theta = arccos(2s-1) = 2*atan(sqrt(1/s-1)), T_k = sin(pi/2 - 2k*atan(.)).
   The "+1" of the reference basis is folded into the weight table.
 - F1 features are feature-major [p, half, q, k, j] and built with one fused
   DVE op per half; PE transposes merge 2 j-columns per instruction, with the
   weight matrix rows permuted to the (feat,jparity)-interleaved order.
 - A block's SEG=4 triplets occupy 4 consecutive columns of one partition, so
   the per-block pre-reduction is 3 DVE adds (no PE matmuls).
"""
import sys

sys.path.insert(0, "/opt/trn_rl_repo")
import numpy as np

from concourse import bass, bacc, mybir, tile
from concourse.bass_utils import run_bass_kernel_spmd

N_TYPES, N_DESC, K_MAX, L_MAX = 4, 8, 8, 4
R_C = 5.0
N_ATOMS = 20000
N_CORES = 8
DL = N_DESC * L_MAX          # 32
QPAD = 20480                 # 128 * 160
STRIDE = 64                  # q row stride in f32 (256B; scatter needs 256B mult)
J = 128                      # field columns per macro-tile
MACRO = 128 * J              # 16384 triplets per macro
KACC = 2                     # rotating DRAM accumulators
CHUNK = 4096                 # max idxs (blocks) per scatter instruction
SEG = 4                      # triplets pre-reduced per scattered block
F32, BF16, I16 = mybir.dt.float32, mybir.dt.bfloat16, mybir.dt.int16
PI = float(np.pi)
CHEB_ATAN = False            # activation-engine Chebyshev (HW Sin/Arctan input
                             # ranges are too narrow; use the DVE recurrence)


def _host_prep(n_atoms, triplet_index, r_ij, r_ik, cos_theta,
               type_i, type_j, type_k, c_table):
    """Shard; group each atom's triplets into SEG-slot blocks (pad slots use
    r=r_c so fc=0 => ang=0); order blocks by block-rank classes; pad to a
    uniform layout across cores.  Block b's slots live at partition b%128,
    columns SEG*(b//128)..+SEG-1 (block-major columns)."""
    T = triplet_index.shape[0]
    Tc = T // N_CORES
    atom_all = np.asarray(triplet_index[:, 0], dtype=np.int64)

    per_core = []
    max_nblk = 0
    for c in range(N_CORES):
        sl = slice(c * Tc, (c + 1) * Tc)
        atom = atom_all[sl]
        order = np.argsort(atom, kind="stable")
        sa = atom[order]
        first = np.r_[True, sa[1:] != sa[:-1]]
        idxf = np.where(first)[0]
        counts = np.diff(np.r_[idxf, Tc])
        uatoms = sa[idxf]
        nblk_per_atom = -(-counts // SEG)
        per_core.append((sl, atom, order, uatoms, counts, nblk_per_atom))
        max_nblk = max(max_nblk, int(nblk_per_atom.max()))

    nclass = max_nblk
    # per class r: number of blocks (padded to 128) -- uniform across cores
    cls_blk = []
    for r in range(nclass):
        mx = max(int(np.sum(nb > r)) for (_, _, _, _, _, nb) in per_core)
        cls_blk.append(-(-max(mx, 1) // 128) * 128)
    total_blk = sum(cls_blk)
    MB = MACRO // SEG                     # blocks per macro
    nmacro = -(-total_blk // MB)
    TBLK = nmacro * MB
    TPAD = TBLK * SEG

    # chunk table in blocks: (start_blk, len_blk); aligned to macros
    chunks = []
    o = 0
    for r in range(nclass):
        p = cls_blk[r]
        sblk = 0
        while sblk < p:
            cl = min(CHUNK, p - sblk)
            mstart = (o + sblk) // MB
            if (o + sblk + cl - 1) // MB != mstart:
                cl = (mstart + 1) * MB - (o + sblk)
            chunks.append((o + sblk, cl))
            sblk += cl
        o += p
    tail = o
    while tail < TBLK:
        cl = min(CHUNK, TBLK - tail, ((tail // MB) + 1) * MB - tail)
        chunks.append((tail, cl))
        tail += cl

    G = TPAD // 128
    cores = []
    for c in range(N_CORES):
        sl, atom, order, uatoms, counts, nblk = per_core[c]
        f32 = lambda x: np.asarray(x[sl], dtype=np.float32)
        fields = dict(r_ij=f32(r_ij), r_ik=f32(r_ik), ct=f32(cos_theta),
                      ti=f32(type_i), tj=f32(type_j), tk=f32(type_k))
        # pad slots: r=R_C -> fc=0 -> ang=0 exactly
        dev = {}
        for n in fields:
            fillv = R_C if n in ("r_ij", "r_ik") else 0.0
            dev[n] = np.full((128, G), fillv, dtype=np.float32)
        bidx = np.empty(TBLK, dtype=np.int16)
        bidx[:] = (20000 + (np.arange(TBLK, dtype=np.int64) % 480)).astype(np.int16)

        # block table: for each unique atom, blocks 0..nblk-1; block b holds
        # triplets order[start + b*SEG : ...] (clipped); class-major placement
        starts = np.r_[0, np.cumsum(counts)[:-1]]
        o = 0
        for r in range(nclass):
            members = np.where(nblk > r)[0]         # atoms having block r
            if len(members):
                a_st = starts[members] + r * SEG
                a_ct = np.minimum(counts[members] - r * SEG, SEG)
                blk = o + np.arange(len(members))   # destination block ids
                src_idx = []
                dst_p = []
                dst_c = []
                for st, ctn, b in zip(a_st, a_ct, blk):
                    src_idx.append(order[st:st + ctn])
                    dst_p.append(np.full(ctn, b % 128))
                    dst_c.append(SEG * (b // 128) + np.arange(ctn))
                src_idx = np.concatenate(src_idx)
                dst_p = np.concatenate(dst_p)
                dst_c = np.concatenate(dst_c)
                for n in dev:
                    dev[n][dst_p, dst_c] = fields[n][src_idx]
                bidx[o:o + len(members)] = uatoms[members].astype(np.int16)
            o += cls_blk[r]
        arrays = {n: dev[n] for n in dev}
        # idx wrapped [16, TBLK/16] and replicated over the 8 gpsimd cores
        arrays["idx"] = np.tile(bidx.reshape(TBLK // 16, 16).T, (8, 1)).copy()
        cores.append(arrays)

    # weight table with the (T_k + 1) -> T_k fold:
    #   sum_k c[d,k]*(T_k+1)*u = sum_k c'[d,k]*T_k*u,  c'[d,0] += sum_k c[d,k]
    ctab = np.asarray(c_table, dtype=np.float64).copy()
    ctab[:, :, :, 0] += ctab.sum(axis=3)
    ctab = ctab.astype(np.float32)
    W = np.zeros((32, 32), dtype=np.float32)
    for tjv in range(4):
        for k in range(8):
            W[tjv * 8 + k, :] = ctab[:, tjv, :, k].reshape(32)
    W4 = np.zeros((128, 128), dtype=np.float32)
    for b in range(4):
        W4[b * 32:(b + 1) * 32, b * 32:(b + 1) * 32] = W
    # permute rows to the (feat, jparity)-interleaved transpose output order:
    # new row (2f+jj) = old row (jj*64+f)
    W4p = np.zeros_like(W4)
    for jj in range(2):
        for f in range(64):
            W4p[2 * f + jj, :] = W4[jj * 64 + f, :]
    consts = dict(w4=W4p, ident=np.eye(128, dtype=np.float32),
                  iota4=np.tile(np.arange(4, dtype=np.float32), (128, 1)))
    return cores, consts, chunks, nmacro, TPAD


def _build(chunks, nmacro, TPAD, debug=False):
    G = TPAD // 128
    MB = MACRO // SEG
    nc = bacc.Bacc(None, target_bir_lowering=False, num_devices=N_CORES,
                   dynamic_dma_scratch_size=32768, num_swdge_queues=2)
    dbg = {}
    if debug:
        dbg["f1"] = nc.declare_dram_parameter("dbg_f1", [128, 64 * J], F32,
                                              isOutput=True)
        dbg["prodc"] = nc.declare_dram_parameter("dbg_prodc", [128, J * 32], F32,
                                                 isOutput=True)
        dbg["partials"] = nc.declare_dram_parameter("dbg_partials", [128, 32 * 32],
                                                    F32, isOutput=True)
    P = {}
    for n in ("r_ij", "r_ik", "ct", "ti", "tj", "tk"):
        P[n] = nc.declare_dram_parameter(n, [128, G], F32, isOutput=False)
    P["idx"] = nc.declare_dram_parameter("idx", [128, TPAD // SEG // 16], I16,
                                         isOutput=False)
    P["w4"] = nc.declare_dram_parameter("w4", [128, 128], F32, isOutput=False)
    P["ident"] = nc.declare_dram_parameter("ident", [128, 128], F32, isOutput=False)
    P["iota4"] = nc.declare_dram_parameter("iota4", [128, 4], F32, isOutput=False)
    out_d = nc.declare_dram_parameter("out", [N_ATOMS, DL], F32, isOutput=True)

    qacc = [nc.dram_tensor(f"qacc{k}", [QPAD, STRIDE], F32) for k in range(KACC)]
    bounce_in = nc.dram_tensor("bounce_in", [128, QPAD * DL // 128], F32)
    bounce_out = nc.dram_tensor("bounce_out", [128, QPAD * DL // 128], F32,
                                addr_space="Shared")

    AF = mybir.ActivationFunctionType
    OP = mybir.AluOpType

    with tile.TileContext(nc) as tc:
        with tc.tile_pool(name="const", bufs=1) as cst:
            w4 = cst.tile([128, 128], BF16)
            ident = cst.tile([128, 128], BF16)
            iota4 = cst.tile([128, 4], F32)
            tmpf = cst.tile([128, 128], F32)
            zero = cst.tile([128, 2048], F32)
            halfpi = cst.tile([128, 1], F32)
            nc.vector.memset(halfpi[:], PI / 2)
            negone = cst.tile([128, 1], F32)
            nc.vector.memset(negone[:], -1.0)
            dbg_t = cst.tile([128, 1024], F32, name="dbg_t") if debug else None
            nc.sync.dma_start(out=tmpf[:], in_=P["w4"][:])
            nc.vector.tensor_copy(out=w4[:], in_=tmpf[:])
            nc.sync.dma_start(out=tmpf[:], in_=P["ident"][:])
            nc.vector.tensor_copy(out=ident[:], in_=tmpf[:])
            nc.sync.dma_start(out=iota4[:], in_=P["iota4"][:])
            nc.vector.memset(zero[:], 0.0)
            qf0 = qacc[0].ap().rearrange("(p r) s -> p (r s)", p=128)
            w0 = QPAD * STRIDE // 128
            for i in range(0, w0, 2048):
                nc.sync.dma_start(out=qf0[:, i:i + 2048],
                                  in_=zero[:, :min(2048, w0 - i)])

            with (
                tc.tile_pool(name="fields", bufs=3) as fpool,
                tc.tile_pool(name="idxp", bufs=3) as ipool,
                tc.tile_pool(name="work", bufs=2) as wpool,
                tc.tile_pool(name="f1p", bufs=3) as f1pool,
                tc.tile_pool(name="prp", bufs=3) as prpool,
                tc.tile_pool(name="cinp", bufs=4) as cpool,
                tc.tile_pool(name="angp", bufs=2) as apool,
                tc.tile_pool(name="scat", bufs=3) as spool,
                tc.tile_pool(name="ps1", bufs=3, space="PSUM") as ppool1,
                tc.tile_pool(name="ps2", bufs=3, space="PSUM") as ppool2,
            ):
                by_macro = [[] for _ in range(nmacro)]
                for ci, (s, pl) in enumerate(chunks):
                    by_macro[s // MB].append((ci, s % MB, pl))

                for m in range(nmacro):
                    fld = {}
                    for n in ("r_ij", "r_ik", "ct", "ti", "tj", "tk"):
                        t = fpool.tile([128, J], F32)
                        nc.sync.dma_start(out=t[:], in_=P[n][:, m * J:(m + 1) * J])
                        fld[n] = t
                    NIC = MB // 16   # idx cols per macro
                    idxs = ipool.tile([128, NIC], I16)
                    nc.sync.dma_start(out=idxs[:],
                                      in_=P["idx"][:, m * NIC:(m + 1) * NIC])
                    if m == 0:
                        # qacc1 zeroing rides behind macro-0 loads; its first
                        # scatter is a macro later (qacc0 was zeroed upfront)
                        for k in range(1, KACC):
                            qf = qacc[k].ap().rearrange("(p r) s -> p (r s)", p=128)
                            w = QPAD * STRIDE // 128
                            for i in range(0, w, 2048):
                                nc.sync.dma_start(out=qf[:, i:i + 2048],
                                                  in_=zero[:, :min(2048, w - i)])

                    # ---- bases + one-hot expanded features ----
                    # layout [p, jpair, feat=(h,q,k), jparity]: a j-pair's 128
                    # features are contiguous -> one full-width PE transpose
                    F1 = f1pool.tile([128, J // 2, 64, 2], BF16)
                    for half, (rn, tn) in enumerate((("r_ij", "tj"), ("r_ik", "tk"))):
                        r = fld[rn]
                        u = wpool.tile([128, J], F32)
                        nc.scalar.activation(u[:], r[:], AF.Sin,
                                             bias=halfpi[:], scale=-PI / R_C)
                        nc.vector.tensor_scalar(out=u[:], in0=u[:], scalar1=0.25,
                                                scalar2=0.25, op0=OP.mult, op1=OP.add)
                        ub = wpool.tile([128, J], BF16)
                        nc.vector.tensor_copy(out=ub[:], in_=u[:])
                        s = wpool.tile([128, J], F32)
                        nc.scalar.activation(s[:], r[:], AF.Square,
                                             bias=negone[:], scale=1.0 / R_C)
                        x = wpool.tile([128, J], F32)
                        nc.vector.tensor_scalar(out=x[:], in0=s[:], scalar1=2.0,
                                                scalar2=-1.0, op0=OP.mult,
                                                op1=OP.add)
                        x2 = wpool.tile([128, J], F32)
                        nc.vector.tensor_scalar_mul(x2[:], x[:], 2.0)
                        Traw = wpool.tile([128, 8, J], F32)
                        nc.vector.memset(Traw[:, 0, :], 1.0)
                        nc.vector.tensor_copy(out=Traw[:, 1, :], in_=x[:])
                        for k in range(2, 8):
                            nc.vector.tensor_tensor(out=Traw[:, k, :], in0=x2[:],
                                                    in1=Traw[:, k - 1, :],
                                                    op=OP.mult)
                            nc.vector.tensor_tensor(out=Traw[:, k, :],
                                                    in0=Traw[:, k, :],
                                                    in1=Traw[:, k - 2, :],
                                                    op=OP.subtract)
                        # Tu[jp, k, jj] = T_k(2jp+jj) * u(2jp+jj)  (pair-contig)
                        Tu = wpool.tile([128, J // 2, 8, 2], BF16)
                        Trs = Traw[:].rearrange("p k (jp jj) -> p jp k jj", jj=2)
                        ubs = ub[:].rearrange("p (jp jj) -> p jp jj", jj=2)
                        nc.vector.tensor_tensor(
                            out=Tu[:], in0=Trs,
                            in1=ubs.unsqueeze(2).broadcast_to([128, J // 2, 8, 2]),
                            op=OP.mult)
                        # oh[jp, q, jj] = (tq(2jp+jj) == q)  (pair-contig)
                        oh = wpool.tile([128, J // 2, 4, 2], BF16)
                        tq = fld[tn]
                        tqs = tq[:].rearrange("p (jp jj) -> p jp jj", jj=2)
                        nc.vector.tensor_tensor(
                            out=oh[:],
                            in0=tqs.unsqueeze(2).broadcast_to([128, J // 2, 4, 2]),
                            in1=iota4[:].unsqueeze(1).unsqueeze(3)
                                .broadcast_to([128, J // 2, 4, 2]),
                            op=OP.is_equal)
                        # F1[jp, (half,q,k), jj] = Tu[jp, k, jj] * oh[jp, q, jj]
                        Fv = F1[:].rearrange("p jp (h q k) jj -> p h jp q k jj",
                                             h=2, q=4)
                        for jj in range(2):
                            nc.vector.tensor_tensor(
                                out=Fv[:, half, :, :, :, jj],
                                in0=Tu[:, :, :, jj].unsqueeze(2)
                                    .broadcast_to([128, J // 2, 4, 8]),
                                in1=oh[:, :, :, jj].unsqueeze(3)
                                    .broadcast_to([128, J // 2, 4, 8]),
                                op=OP.mult)

                    oh_ti = wpool.tile([128, J, 4], BF16)
                    nc.vector.tensor_tensor(
                        out=oh_ti[:],
                        in0=fld["ti"][:].unsqueeze(2).broadcast_to([128, J, 4]),
                        in1=iota4[:].unsqueeze(1).broadcast_to([128, J, 4]),
                        op=OP.is_equal)

                    # Legendre (bf16): P0=1, P1=ct, P2=1.5ct^2-.5, P3=ct(2.5ct^2-1.5)
                    ctf = fld["ct"]
                    P4 = wpool.tile([128, J, 4], BF16)
                    ct2 = wpool.tile([128, J], F32)
                    nc.vector.tensor_tensor(out=ct2[:], in0=ctf[:], in1=ctf[:],
                                            op=OP.mult)
                    nc.vector.memset(P4[:, :, 0], 1.0)
                    nc.vector.tensor_copy(out=P4[:, :, 1], in_=ctf[:])
                    nc.vector.tensor_scalar(out=P4[:, :, 2], in0=ct2[:], scalar1=1.5,
                                            scalar2=-0.5, op0=OP.mult, op1=OP.add)
                    p3t = wpool.tile([128, J], F32)
                    nc.vector.tensor_scalar(out=p3t[:], in0=ct2[:], scalar1=2.5,
                                            scalar2=-1.5, op0=OP.mult, op1=OP.add)
                    nc.vector.tensor_tensor(out=p3t[:], in0=p3t[:], in1=ctf[:],
                                            op=OP.mult)
                    nc.vector.tensor_copy(out=P4[:, :, 3], in_=p3t[:])

                    # ---- PE: per 8-column group ----
                    prodc = prpool.tile([128, J, 32], BF16)
                    F1m = F1[:].rearrange("p jp f jj -> p jp (f jj)")
                    NG = J // 8
                    for grp in range(NG):
                        j0 = grp * 8
                        ps1 = ppool1.tile([128, 4, 128], BF16, space="PSUM")
                        for c in range(4):
                            nc.tensor.transpose(
                                out=ps1[:, c, :], in_=F1m[:, 4 * grp + c, :],
                                identity=ident[:])
                        cin = cpool.tile([128, 4, 128], BF16)
                        nc.scalar.activation(
                            cin[:].rearrange("p c f -> p (c f)"),
                            ps1[:].rearrange("p c f -> p (c f)"), AF.Identity)
                        ps2 = ppool2.tile([128, 4, 128], F32, space="PSUM")
                        for c in range(4):
                            nc.tensor.matmul(out=ps2[:, c, :], lhsT=cin[:, c, :],
                                             rhs=w4[:], start=True, stop=True)
                        cnd = cpool.tile([128, 4, 128], BF16)
                        nc.scalar.activation(
                            cnd[:].rearrange("p c f -> p (c f)"),
                            ps2[:].rearrange("p c f -> p (c f)"), AF.Identity)
                        # prodc[j0+2c+jj, f] = g_ij * g_ik
                        psv = cnd[:].rearrange("p c (j h f) -> p c j h f", j=2, h=2)
                        nc.vector.tensor_tensor(
                            out=prodc[:, j0:j0 + 8, :]
                                .rearrange("p (c j) f -> p c j f", c=4),
                            in0=psv[:, :, :, 0, :], in1=psv[:, :, :, 1, :],
                            op=OP.mult)

                    # ---- ti select: h = sum_q ohti[q] * prodc[:, :, q*8:(q+1)*8]
                    h = wpool.tile([128, J, 8], BF16)
                    t0 = wpool.tile([128, J, 8], BF16)
                    nc.vector.tensor_tensor(
                        out=h[:], in0=prodc[:, :, 0:8],
                        in1=oh_ti[:, :, 0].unsqueeze(2).broadcast_to([128, J, 8]),
                        op=OP.mult)
                    for q in range(1, 4):
                        nc.vector.tensor_tensor(
                            out=t0[:], in0=prodc[:, :, q * 8:(q + 1) * 8],
                            in1=oh_ti[:, :, q].unsqueeze(2).broadcast_to([128, J, 8]),
                            op=OP.mult)
                        nc.vector.tensor_tensor(out=h[:], in0=h[:], in1=t0[:],
                                                op=OP.add)

                    # ---- ang = h (x) P (bf16) ----
                    ang = apool.tile([128, J, 8, 4], BF16)
                    nc.vector.tensor_tensor(
                        out=ang[:],
                        in0=h[:].unsqueeze(3).broadcast_to([128, J, 8, 4]),
                        in1=P4[:].unsqueeze(2).broadcast_to([128, J, 8, 4]),
                        op=OP.mult)

                    # ---- per-block pre-reduction: 3 adds over 4 columns ----
                    partials = spool.tile([128, J // SEG, 32], F32)
                    av = ang[:].rearrange("p (g s) d l -> p g s (d l)", s=SEG)
                    ts1 = wpool.tile([128, J // SEG, 32], BF16)
                    ts2 = wpool.tile([128, J // SEG, 32], BF16)
                    nc.vector.tensor_tensor(out=ts1[:], in0=av[:, :, 0, :],
                                            in1=av[:, :, 1, :], op=OP.add)
                    nc.vector.tensor_tensor(out=ts2[:], in0=av[:, :, 2, :],
                                            in1=av[:, :, 3, :], op=OP.add)
                    nc.vector.tensor_tensor(out=partials[:], in0=ts1[:],
                                            in1=ts2[:], op=OP.add)

                    if debug and m == 0:
                        for nm, src, w in (
                            ("f1", F1[:].rearrange("p jp f jj -> p (jp f jj)"), 64 * J),
                            ("prodc", prodc[:].rearrange("p j f -> p (j f)"), J * 32),
                            ("partials", partials[:].rearrange("p g f -> p (g f)"),
                             32 * 32),
                        ):
                            for i in range(0, w, 1024):
                                e = min(i + 1024, w)
                                nc.vector.tensor_copy(out=dbg_t[:, :e - i],
                                                      in_=src[:, i:e])
                                nc.sync.dma_start(out=dbg[nm].ap()[:, i:e],
                                                  in_=dbg_t[:, :e - i])

                    # ---- scatter chunks of this macro (block granularity) ----
                    for (ci, off, pl) in by_macro[m]:
                        nc.gpsimd.dma_scatter_add(
                            qacc[ci % KACC].ap()[:, :DL],
                            partials[:, off // 128:(off + pl) // 128, :],
                            idxs[:, off // 16:(off + pl) // 16],
                            pl, pl, DL, elem_step=STRIDE,
                            queue_num=ci % 2)

            # ---- reduce K accumulators, AllReduce, emit ----
            with tc.tile_pool(name="red", bufs=1) as rpool:
                W = QPAD * STRIDE // 128   # 10240
                acc = rpool.tile([128, W], F32)
                tmp = rpool.tile([128, W], F32)
                qv = [q.ap().rearrange("(p r) s -> p (r s)", p=128) for q in qacc]
                nc.sync.dma_start(out=acc[:], in_=qv[0])
                nc.sync.dma_start(out=tmp[:], in_=qv[1])
                nc.vector.tensor_tensor(out=acc[:], in0=acc[:], in1=tmp[:], op=OP.add)
                packed = rpool.tile([128, QPAD * DL // 128], F32)  # [128, 5120]
                nc.vector.tensor_copy(
                    out=packed[:].rearrange("p (r s) -> p r s", s=DL),
                    in_=acc[:].rearrange("p (r s) -> p r s", s=STRIDE)[:, :, :DL])
                nc.sync.dma_start(out=bounce_in.ap(), in_=packed[:])
                nc.gpsimd.collective_compute(
                    "AllReduce", OP.add,
                    replica_groups=[list(range(N_CORES))],
                    ins=[bounce_in.ap().opt()], outs=[bounce_out.ap().opt()])
                nc.sync.dma_start(
                    out=out_d[:, :],
                    in_=bounce_out.ap().rearrange("p f -> (p f)")[
                        :N_ATOMS * DL].rearrange("(a c) -> a c", c=DL))
    nc.compile()
    return nc


def _install_ntff_hook():
    """Provide antenv.axon_hooks (missing in this image) via sys.modules so
    run_bass_kernel_spmd(trace=True) can capture NTFF profiles."""
    import types, ctypes, contextlib
    try:
        from antenv.axon_hooks import get_axon_ntff_profile_hook  # noqa: F401
        return
    except ImportError:
        pass
    so_path = "/opt/axon/libaxon_pjrt.so"
    try:
        lib = ctypes.CDLL(so_path)
    except OSError:
        return
    if not hasattr(lib, "axon_start_nrt_profile"):
        return
    lib.axon_start_nrt_profile.argtypes = [ctypes.POINTER(ctypes.c_int64),
                                           ctypes.c_size_t]
    lib.axon_start_nrt_profile.restype = ctypes.c_int64
    lib.axon_stop_nrt_profile.argtypes = [ctypes.c_char_p]
    lib.axon_stop_nrt_profile.restype = ctypes.c_int64

    @contextlib.contextmanager
    def _hook(output_dir, device_ids):
        import jax
        jax.devices()
        if device_ids:
            ids = (ctypes.c_int64 * len(device_ids))(*device_ids)
            rc = lib.axon_start_nrt_profile(ids, len(device_ids))
        else:
            rc = lib.axon_start_nrt_profile(None, 0)
        if rc != 0:
            raise RuntimeError(f"axon_start_nrt_profile rc={rc}")
        try:
            yield
        finally:
            n = lib.axon_stop_nrt_profile(str(output_dir).encode())
            if n <= 0:
                print(f"ntff capture wrote {n} files", flush=True)

    mod = types.ModuleType("antenv.axon_hooks")
    mod.get_axon_ntff_profile_hook = lambda: _hook
    mod.set_axon_ntff_profile_hook = lambda h: None
    import antenv
    sys.modules["antenv.axon_hooks"] = mod
    antenv.axon_hooks = mod



_CACHE = {}


def kernel(n_atoms, triplet_index, r_ij, r_ik, cos_theta,
           type_i, type_j, type_k, c_table, _sim=False, _trace=False):
    cores, consts, chunks, nmacro, TPAD = _host_prep(
        n_atoms, triplet_index, r_ij, r_ik, cos_theta,
        type_i, type_j, type_k, c_table)
    key = (nmacro, TPAD, tuple(chunks))
    if key not in _CACHE:
        _CACHE[key] = _build(chunks, nmacro, TPAD)
    nc = _CACHE[key]
    in_maps = []
    for c in range(N_CORES):
        m = dict(cores[c])
        m.update(consts)
        in_maps.append(m)
    if _sim:
        from concourse import bass_interp
        sim = bass_interp.MultiCoreSim(nc, N_CORES)
        for c in range(N_CORES):
            for k, v in in_maps[c].items():
                sim.cores[c].tensor(k)[:] = v
        sim.simulate()
        out = np.array(sim.cores[0].mem_tensor("out"))
    else:
        if _trace:
            _install_ntff_hook()
        last_err = None
        for _try in range(3):
            try:
                res = run_bass_kernel_spmd(nc, in_maps,
                                           core_ids=list(range(N_CORES)),
                                           trace=_trace)
                out = np.asarray(res.results[0]["out"])
                break
            except Exception as e:  # transient device-unrecoverable after a crash
                last_err = e
        else:
            raise last_err
        kernel.last_exec_ns = res.exec_time_ns
        kernel.last_results = res
    return out.reshape(N_ATOMS, N_DESC, L_MAX).astype(np.float32)


# revision 34
# speedup vs baseline: 1.3798x; 1.0732x over previous
"""AngularDescriptor Trainium2 kernel (8 NeuronCores, SPMD + AllReduce).

Per core: T/8 triplets.  Device computes Chebyshev/Legendre bases, the
per-pair-type radial einsum (PE matmul with fixed block-diag weights after a
4-way tj/tk one-hot expansion; 4-way ti select on DVE), the outer product
ang = (g_ij*g_ik) (x) P_l, and segment-sums ang into q[20000,8,4] via
gpsimd.dma_scatter_add.  HW scatter-add loses duplicate indices within one
instruction (last-write-wins race), so the host orders each shard's triplets
into occurrence-rank classes (class r = r-th occurrence of an atom): within a
class all atom indices are unique.  Classes are cut into chunks that rotate
over K DRAM accumulators, so same-accumulator scatters serialize (WAW dep)
while different-accumulator scatters overlap.  Padding slots scatter to
distinct dummy atom rows (20000..20479) so one uniform program serves all
cores.  Final: on-device K-way add, AllReduce over the 8 cores, output from
core 0.

v2 layout notes:
 - Chebyshev T_k computed on the Activation engine: with s=(r/rc-1)^2,
   theta = arccos(2s-1) = 2*atan(sqrt(1/s-1)), T_k = sin(pi/2 - 2k*atan(.)).
   The "+1" of the reference basis is folded into the weight table.
 - F1 features are feature-major [p, half, q, k, j] and built with one fused
   DVE op per half; PE transposes merge 2 j-columns per instruction, with the
   weight matrix rows permuted to the (feat,jparity)-interleaved order.
 - A block's SEG=4 triplets occupy 4 consecutive columns of one partition, so
   the per-block pre-reduction is 3 DVE adds (no PE matmuls).
"""
import sys

sys.path.insert(0, "/opt/trn_rl_repo")
import numpy as np

from concourse import bass, bacc, mybir, tile
from concourse.bass_utils import run_bass_kernel_spmd

N_TYPES, N_DESC, K_MAX, L_MAX = 4, 8, 8, 4
R_C = 5.0
N_ATOMS = 20000
N_CORES = 8
DL = N_DESC * L_MAX          # 32
QPAD = 20480                 # 128 * 160
STRIDE = 64                  # q row stride in f32 (256B; scatter needs 256B mult)
J = 128                      # field columns per macro-tile
MACRO = 128 * J              # 16384 triplets per macro
KACC = 2                     # rotating DRAM accumulators
CHUNK = 4096                 # max idxs (blocks) per scatter instruction
SEG = 4                      # triplets pre-reduced per scattered block
F32, BF16, I16 = mybir.dt.float32, mybir.dt.bfloat16, mybir.dt.int16
PI = float(np.pi)
CHEB_ATAN = False            # activation-engine Chebyshev (HW Sin/Arctan input
                             # ranges are too narrow; use the DVE recurrence)


def _host_prep(n_atoms, triplet_index, r_ij, r_ik, cos_theta,
               type_i, type_j, type_k, c_table):
    """Shard; group each atom's triplets into SEG-slot blocks (pad slots use
    r=r_c so fc=0 => ang=0); order blocks by block-rank classes; pad to a
    uniform layout across cores.  Block b's slots live at partition b%128,
    columns SEG*(b//128)..+SEG-1 (block-major columns)."""
    T = triplet_index.shape[0]
    Tc = T // N_CORES
    atom_all = np.asarray(triplet_index[:, 0], dtype=np.int64)

    per_core = []
    max_nblk = 0
    for c in range(N_CORES):
        sl = slice(c * Tc, (c + 1) * Tc)
        atom = atom_all[sl]
        order = np.argsort(atom, kind="stable")
        sa = atom[order]
        first = np.r_[True, sa[1:] != sa[:-1]]
        idxf = np.where(first)[0]
        counts = np.diff(np.r_[idxf, Tc])
        uatoms = sa[idxf]
        nblk_per_atom = -(-counts // SEG)
        per_core.append((sl, atom, order, uatoms, counts, nblk_per_atom))
        max_nblk = max(max_nblk, int(nblk_per_atom.max()))

    nclass = max_nblk
    # per class r: number of blocks (padded to 128) -- uniform across cores
    cls_blk = []
    for r in range(nclass):
        mx = max(int(np.sum(nb > r)) for (_, _, _, _, _, nb) in per_core)
        cls_blk.append(-(-max(mx, 1) // 128) * 128)
    total_blk = sum(cls_blk)
    MB = MACRO // SEG                     # blocks per macro
    nmacro = -(-total_blk // MB)
    TBLK = nmacro * MB
    TPAD = TBLK * SEG

    # chunk table in blocks: (start_blk, len_blk); aligned to macros
    chunks = []
    o = 0
    for r in range(nclass):
        p = cls_blk[r]
        sblk = 0
        while sblk < p:
            cl = min(CHUNK, p - sblk)
            mstart = (o + sblk) // MB
            if (o + sblk + cl - 1) // MB != mstart:
                cl = (mstart + 1) * MB - (o + sblk)
            chunks.append((o + sblk, cl))
            sblk += cl
        o += p
    tail = o
    while tail < TBLK:
        cl = min(CHUNK, TBLK - tail, ((tail // MB) + 1) * MB - tail)
        chunks.append((tail, cl))
        tail += cl

    G = TPAD // 128
    cores = []
    for c in range(N_CORES):
        sl, atom, order, uatoms, counts, nblk = per_core[c]
        f32 = lambda x: np.asarray(x[sl], dtype=np.float32)
        fields = dict(r_ij=f32(r_ij), r_ik=f32(r_ik), ct=f32(cos_theta),
                      ti=f32(type_i), tj=f32(type_j), tk=f32(type_k))
        # pad slots: r=R_C -> fc=0 -> ang=0 exactly
        dev = {}
        for n in fields:
            fillv = R_C if n in ("r_ij", "r_ik") else 0.0
            dev[n] = np.full((128, G), fillv, dtype=np.float32)
        bidx = np.empty(TBLK, dtype=np.int16)
        bidx[:] = (20000 + (np.arange(TBLK, dtype=np.int64) % 480)).astype(np.int16)

        # block table: for each unique atom, blocks 0..nblk-1; block b holds
        # triplets order[start + b*SEG : ...] (clipped); class-major placement
        starts = np.r_[0, np.cumsum(counts)[:-1]]
        o = 0
        for r in range(nclass):
            members = np.where(nblk > r)[0]         # atoms having block r
            if len(members):
                a_st = starts[members] + r * SEG
                a_ct = np.minimum(counts[members] - r * SEG, SEG)
                blk = o + np.arange(len(members))   # destination block ids
                src_idx = []
                dst_p = []
                dst_c = []
                for st, ctn, b in zip(a_st, a_ct, blk):
                    src_idx.append(order[st:st + ctn])
                    dst_p.append(np.full(ctn, b % 128))
                    dst_c.append(SEG * (b // 128) + np.arange(ctn))
                src_idx = np.concatenate(src_idx)
                dst_p = np.concatenate(dst_p)
                dst_c = np.concatenate(dst_c)
                for n in dev:
                    dev[n][dst_p, dst_c] = fields[n][src_idx]
                bidx[o:o + len(members)] = uatoms[members].astype(np.int16)
            o += cls_blk[r]
        arrays = {n: dev[n] for n in dev}
        # idx wrapped [16, TBLK/16] and replicated over the 8 gpsimd cores
        arrays["idx"] = np.tile(bidx.reshape(TBLK // 16, 16).T, (8, 1)).copy()
        cores.append(arrays)

    # weight table with the (T_k + 1) -> T_k fold:
    #   sum_k c[d,k]*(T_k+1)*u = sum_k c'[d,k]*T_k*u,  c'[d,0] += sum_k c[d,k]
    ctab = np.asarray(c_table, dtype=np.float64).copy()
    ctab[:, :, :, 0] += ctab.sum(axis=3)
    ctab = ctab.astype(np.float32)
    W = np.zeros((32, 32), dtype=np.float32)
    for tjv in range(4):
        for k in range(8):
            W[tjv * 8 + k, :] = ctab[:, tjv, :, k].reshape(32)
    W4 = np.zeros((128, 128), dtype=np.float32)
    for b in range(4):
        W4[b * 32:(b + 1) * 32, b * 32:(b + 1) * 32] = W
    # permute rows to the (feat, jparity)-interleaved transpose output order:
    # new row (2f+jj) = old row (jj*64+f)
    W4p = np.zeros_like(W4)
    for jj in range(2):
        for f in range(64):
            W4p[2 * f + jj, :] = W4[jj * 64 + f, :]
    consts = dict(w4=W4p, ident=np.eye(128, dtype=np.float32),
                  iota4=np.tile(np.arange(4, dtype=np.float32), (128, 1)))
    return cores, consts, chunks, nmacro, TPAD


def _build(chunks, nmacro, TPAD, debug=False):
    G = TPAD // 128
    MB = MACRO // SEG
    nc = bacc.Bacc(None, target_bir_lowering=False, num_devices=N_CORES,
                   dynamic_dma_scratch_size=32768, num_swdge_queues=2)
    dbg = {}
    if debug:
        dbg["f1"] = nc.declare_dram_parameter("dbg_f1", [128, 64 * J], F32,
                                              isOutput=True)
        dbg["prodc"] = nc.declare_dram_parameter("dbg_prodc", [128, J * 32], F32,
                                                 isOutput=True)
        dbg["partials"] = nc.declare_dram_parameter("dbg_partials", [128, 32 * 32],
                                                    F32, isOutput=True)
    P = {}
    for n in ("r_ij", "r_ik", "ct", "ti", "tj", "tk"):
        P[n] = nc.declare_dram_parameter(n, [128, G], F32, isOutput=False)
    P["idx"] = nc.declare_dram_parameter("idx", [128, TPAD // SEG // 16], I16,
                                         isOutput=False)
    P["w4"] = nc.declare_dram_parameter("w4", [128, 128], F32, isOutput=False)
    P["ident"] = nc.declare_dram_parameter("ident", [128, 128], F32, isOutput=False)
    P["iota4"] = nc.declare_dram_parameter("iota4", [128, 4], F32, isOutput=False)
    out_d = nc.declare_dram_parameter("out", [N_ATOMS, DL], F32, isOutput=True)

    qacc = [nc.dram_tensor(f"qacc{k}", [QPAD, STRIDE], F32) for k in range(KACC)]
    bounce_in = nc.dram_tensor("bounce_in", [128, QPAD * DL // 128], F32)
    bounce_out = nc.dram_tensor("bounce_out", [128, QPAD * DL // 128], F32,
                                addr_space="Shared")

    AF = mybir.ActivationFunctionType
    OP = mybir.AluOpType

    with tile.TileContext(nc) as tc:
        with tc.tile_pool(name="const", bufs=1) as cst:
            w4 = cst.tile([128, 128], BF16)
            ident = cst.tile([128, 128], BF16)
            iota4 = cst.tile([128, 4], F32)
            tmpf = cst.tile([128, 128], F32)
            zero = cst.tile([128, 2048], F32)
            halfpi = cst.tile([128, 1], F32)
            nc.vector.memset(halfpi[:], PI / 2)
            negone = cst.tile([128, 1], F32)
            nc.vector.memset(negone[:], -1.0)
            dbg_t = cst.tile([128, 1024], F32, name="dbg_t") if debug else None
            nc.sync.dma_start(out=tmpf[:], in_=P["w4"][:])
            nc.vector.tensor_copy(out=w4[:], in_=tmpf[:])
            nc.sync.dma_start(out=tmpf[:], in_=P["ident"][:])
            nc.vector.tensor_copy(out=ident[:], in_=tmpf[:])
            nc.sync.dma_start(out=iota4[:], in_=P["iota4"][:])
            nc.vector.memset(zero[:], 0.0)
            qf0 = qacc[0].ap().rearrange("(p r) s -> p (r s)", p=128)
            w0 = QPAD * STRIDE // 128
            for i in range(0, w0, 2048):
                nc.sync.dma_start(out=qf0[:, i:i + 2048],
                                  in_=zero[:, :min(2048, w0 - i)])

            with (
                tc.tile_pool(name="fields", bufs=3) as fpool,
                tc.tile_pool(name="idxp", bufs=3) as ipool,
                tc.tile_pool(name="work", bufs=2) as wpool,
                tc.tile_pool(name="f1p", bufs=3) as f1pool,
                tc.tile_pool(name="prp", bufs=3) as prpool,
                tc.tile_pool(name="cinp", bufs=4) as cpool,
                tc.tile_pool(name="angp", bufs=2) as apool,
                tc.tile_pool(name="scat", bufs=3) as spool,
                tc.tile_pool(name="ps1", bufs=3, space="PSUM") as ppool1,
                tc.tile_pool(name="ps2", bufs=3, space="PSUM") as ppool2,
            ):
                by_macro = [[] for _ in range(nmacro)]
                for ci, (s, pl) in enumerate(chunks):
                    by_macro[s // MB].append((ci, s % MB, pl))
                NIC = MB // 16   # idx cols per macro

                def issue_loads(m):
                    """Field/idx DMAs + the u/s activations for macro m.
                    Called one iteration ahead so the scalar-queue Sin/Square
                    land before the previous macro's cin/cnd copies."""
                    fld = {}
                    for n in ("r_ij", "r_ik", "ct", "ti", "tj", "tk"):
                        t = fpool.tile([128, J], F32, name=f"fld_{n}")
                        nc.sync.dma_start(out=t[:], in_=P[n][:, m * J:(m + 1) * J])
                        fld[n] = t
                    idxs = ipool.tile([128, NIC], I16, name="idxs")
                    nc.sync.dma_start(out=idxs[:],
                                      in_=P["idx"][:, m * NIC:(m + 1) * NIC])
                    us = []
                    for half, rn in enumerate(("r_ij", "r_ik")):
                        r = fld[rn]
                        u = wpool.tile([128, J], F32, name=f"u{half}")
                        nc.scalar.activation(u[:], r[:], AF.Sin,
                                             bias=halfpi[:], scale=-PI / R_C)
                        s = wpool.tile([128, J], F32, name=f"s{half}")
                        nc.scalar.activation(s[:], r[:], AF.Square,
                                             bias=negone[:], scale=1.0 / R_C)
                        us.append((u, s))
                    return fld, idxs, us

                pre = issue_loads(0)
                for k in range(1, KACC):
                    # qacc1 zeroing rides behind macro-0 loads; its first
                    # scatter is a macro later (qacc0 was zeroed upfront)
                    qf = qacc[k].ap().rearrange("(p r) s -> p (r s)", p=128)
                    w = QPAD * STRIDE // 128
                    for i in range(0, w, 2048):
                        nc.sync.dma_start(out=qf[:, i:i + 2048],
                                          in_=zero[:, :min(2048, w - i)])

                for m in range(nmacro):
                    fld, idxs, us = pre
                    if m + 1 < nmacro:
                        pre = issue_loads(m + 1)

                    # ---- bases + one-hot expanded features ----
                    # layout [p, jpair, feat=(h,q,k), jparity]: a j-pair's 128
                    # features are contiguous -> one full-width PE transpose
                    F1 = f1pool.tile([128, J // 2, 64, 2], BF16)
                    Fv = F1[:].rearrange("p jp (h q k) jj -> p h jp q k jj",
                                         h=2, q=4)
                    hx = []
                    for half in range(2):
                        u, s = us[half]
                        nc.vector.tensor_scalar(out=u[:], in0=u[:], scalar1=0.25,
                                                scalar2=0.25, op0=OP.mult,
                                                op1=OP.add)
                        x = wpool.tile([128, J], F32, name=f"x{half}")
                        nc.vector.tensor_scalar(out=x[:], in0=s[:], scalar1=2.0,
                                                scalar2=-1.0, op0=OP.mult,
                                                op1=OP.add)
                        x2 = wpool.tile([128, J], F32, name=f"x2{half}")
                        nc.vector.tensor_scalar_mul(x2[:], x[:], 2.0)
                        Traw = wpool.tile([128, 8, J], F32, name=f"Traw{half}")
                        nc.vector.memset(Traw[:, 0, :], 1.0)
                        nc.vector.tensor_copy(out=Traw[:, 1, :], in_=x[:])
                        hx.append((u, x2, Traw))
                    # interleaved recurrence: the two chains hide each other's
                    # SBUF write-ack latency
                    for k in range(2, 8):
                        for half in range(2):
                            u, x2, Traw = hx[half]
                            nc.vector.tensor_tensor(out=Traw[:, k, :], in0=x2[:],
                                                    in1=Traw[:, k - 1, :],
                                                    op=OP.mult)
                            nc.vector.tensor_tensor(out=Traw[:, k, :],
                                                    in0=Traw[:, k, :],
                                                    in1=Traw[:, k - 2, :],
                                                    op=OP.subtract)
                    for half, tn in enumerate(("tj", "tk")):
                        u, x2, Traw = hx[half]
                        # Tu[jp, k, jj] = T_k(2jp+jj) * u(2jp+jj)  (pair-contig)
                        Tu = wpool.tile([128, J // 2, 8, 2], BF16,
                                        name=f"Tu{half}")
                        Trs = Traw[:].rearrange("p k (jp jj) -> p jp k jj", jj=2)
                        ubs = u[:].rearrange("p (jp jj) -> p jp jj", jj=2)
                        nc.vector.tensor_tensor(
                            out=Tu[:], in0=Trs,
                            in1=ubs.unsqueeze(2).broadcast_to([128, J // 2, 8, 2]),
                            op=OP.mult)
                        # oh[jp, q, jj] = (tq(2jp+jj) == q)  (pair-contig)
                        oh = wpool.tile([128, J // 2, 4, 2], BF16,
                                        name=f"oh{half}")
                        tq = fld[tn]
                        tqs = tq[:].rearrange("p (jp jj) -> p jp jj", jj=2)
                        nc.vector.tensor_tensor(
                            out=oh[:],
                            in0=tqs.unsqueeze(2).broadcast_to([128, J // 2, 4, 2]),
                            in1=iota4[:].unsqueeze(1).unsqueeze(3)
                                .broadcast_to([128, J // 2, 4, 2]),
                            op=OP.is_equal)
                        # F1[jp, (half,q,k), jj] = Tu[jp, k, jj] * oh[jp, q, jj]
                        for jj in range(2):
                            nc.vector.tensor_tensor(
                                out=Fv[:, half, :, :, :, jj],
                                in0=Tu[:, :, :, jj].unsqueeze(2)
                                    .broadcast_to([128, J // 2, 4, 8]),
                                in1=oh[:, :, :, jj].unsqueeze(3)
                                    .broadcast_to([128, J // 2, 4, 8]),
                                op=OP.mult)

                    oh_ti = wpool.tile([128, J, 4], BF16)
                    nc.vector.tensor_tensor(
                        out=oh_ti[:],
                        in0=fld["ti"][:].unsqueeze(2).broadcast_to([128, J, 4]),
                        in1=iota4[:].unsqueeze(1).broadcast_to([128, J, 4]),
                        op=OP.is_equal)

                    # Legendre (bf16): P0=1, P1=ct, P2=1.5ct^2-.5, P3=ct(2.5ct^2-1.5)
                    ctf = fld["ct"]
                    P4 = wpool.tile([128, J, 4], BF16)
                    ct2 = wpool.tile([128, J], F32)
                    nc.vector.tensor_tensor(out=ct2[:], in0=ctf[:], in1=ctf[:],
                                            op=OP.mult)
                    nc.vector.memset(P4[:, :, 0], 1.0)
                    nc.vector.tensor_copy(out=P4[:, :, 1], in_=ctf[:])
                    nc.vector.tensor_scalar(out=P4[:, :, 2], in0=ct2[:], scalar1=1.5,
                                            scalar2=-0.5, op0=OP.mult, op1=OP.add)
                    p3t = wpool.tile([128, J], F32)
                    nc.vector.tensor_scalar(out=p3t[:], in0=ct2[:], scalar1=2.5,
                                            scalar2=-1.5, op0=OP.mult, op1=OP.add)
                    nc.vector.tensor_tensor(out=p3t[:], in0=p3t[:], in1=ctf[:],
                                            op=OP.mult)
                    nc.vector.tensor_copy(out=P4[:, :, 3], in_=p3t[:])

                    # ---- PE: per 8-column group ----
                    prodc = prpool.tile([128, J, 32], BF16)
                    F1m = F1[:].rearrange("p jp f jj -> p jp (f jj)")
                    NG = J // 8
                    for grp in range(NG):
                        j0 = grp * 8
                        ps1 = ppool1.tile([128, 4, 128], BF16, space="PSUM")
                        for c in range(4):
                            nc.tensor.transpose(
                                out=ps1[:, c, :], in_=F1m[:, 4 * grp + c, :],
                                identity=ident[:])
                        cin = cpool.tile([128, 4, 128], BF16)
                        nc.scalar.activation(
                            cin[:].rearrange("p c f -> p (c f)"),
                            ps1[:].rearrange("p c f -> p (c f)"), AF.Identity)
                        ps2 = ppool2.tile([128, 4, 128], F32, space="PSUM")
                        for c in range(4):
                            nc.tensor.matmul(out=ps2[:, c, :], lhsT=cin[:, c, :],
                                             rhs=w4[:], start=True, stop=True)
                        cnd = cpool.tile([128, 4, 128], BF16)
                        nc.scalar.activation(
                            cnd[:].rearrange("p c f -> p (c f)"),
                            ps2[:].rearrange("p c f -> p (c f)"), AF.Identity)
                        # prodc[j0+2c+jj, f] = g_ij * g_ik
                        psv = cnd[:].rearrange("p c (j h f) -> p c j h f", j=2, h=2)
                        nc.vector.tensor_tensor(
                            out=prodc[:, j0:j0 + 8, :]
                                .rearrange("p (c j) f -> p c j f", c=4),
                            in0=psv[:, :, :, 0, :], in1=psv[:, :, :, 1, :],
                            op=OP.mult)

                    # ---- ti select: h = sum_q ohti[q] * prodc[:, :, q*8:(q+1)*8]
                    h = wpool.tile([128, J, 8], BF16)
                    t0 = wpool.tile([128, J, 8], BF16)
                    nc.vector.tensor_tensor(
                        out=h[:], in0=prodc[:, :, 0:8],
                        in1=oh_ti[:, :, 0].unsqueeze(2).broadcast_to([128, J, 8]),
                        op=OP.mult)
                    for q in range(1, 4):
                        nc.vector.tensor_tensor(
                            out=t0[:], in0=prodc[:, :, q * 8:(q + 1) * 8],
                            in1=oh_ti[:, :, q].unsqueeze(2).broadcast_to([128, J, 8]),
                            op=OP.mult)
                        nc.vector.tensor_tensor(out=h[:], in0=h[:], in1=t0[:],
                                                op=OP.add)

                    # ---- ang = h (x) P (bf16) ----
                    ang = apool.tile([128, J, 8, 4], BF16)
                    nc.vector.tensor_tensor(
                        out=ang[:],
                        in0=h[:].unsqueeze(3).broadcast_to([128, J, 8, 4]),
                        in1=P4[:].unsqueeze(2).broadcast_to([128, J, 8, 4]),
                        op=OP.mult)

                    # ---- per-block pre-reduction: 3 adds over 4 columns ----
                    partials = spool.tile([128, J // SEG, 32], F32)
                    av = ang[:].rearrange("p (g s) d l -> p g s (d l)", s=SEG)
                    ts1 = wpool.tile([128, J // SEG, 32], BF16)
                    ts2 = wpool.tile([128, J // SEG, 32], BF16)
                    nc.vector.tensor_tensor(out=ts1[:], in0=av[:, :, 0, :],
                                            in1=av[:, :, 1, :], op=OP.add)
                    nc.vector.tensor_tensor(out=ts2[:], in0=av[:, :, 2, :],
                                            in1=av[:, :, 3, :], op=OP.add)
                    nc.vector.tensor_tensor(out=partials[:], in0=ts1[:],
                                            in1=ts2[:], op=OP.add)

                    if debug and m == 0:
                        for nm, src, w in (
                            ("f1", F1[:].rearrange("p jp f jj -> p (jp f jj)"), 64 * J),
                            ("prodc", prodc[:].rearrange("p j f -> p (j f)"), J * 32),
                            ("partials", partials[:].rearrange("p g f -> p (g f)"),
                             32 * 32),
                        ):
                            for i in range(0, w, 1024):
                                e = min(i + 1024, w)
                                nc.vector.tensor_copy(out=dbg_t[:, :e - i],
                                                      in_=src[:, i:e])
                                nc.sync.dma_start(out=dbg[nm].ap()[:, i:e],
                                                  in_=dbg_t[:, :e - i])

                    # ---- scatter chunks of this macro (block granularity) ----
                    for (ci, off, pl) in by_macro[m]:
                        nc.gpsimd.dma_scatter_add(
                            qacc[ci % KACC].ap()[:, :DL],
                            partials[:, off // 128:(off + pl) // 128, :],
                            idxs[:, off // 16:(off + pl) // 16],
                            pl, pl, DL, elem_step=STRIDE,
                            queue_num=ci % 2)

            # ---- reduce K accumulators, AllReduce, emit ----
            with tc.tile_pool(name="red", bufs=1) as rpool:
                W = QPAD * STRIDE // 128   # 10240
                acc = rpool.tile([128, W], F32)
                tmp = rpool.tile([128, W], F32)
                qv = [q.ap().rearrange("(p r) s -> p (r s)", p=128) for q in qacc]
                nc.sync.dma_start(out=acc[:], in_=qv[0])
                nc.sync.dma_start(out=tmp[:], in_=qv[1])
                nc.vector.tensor_tensor(out=acc[:], in0=acc[:], in1=tmp[:], op=OP.add)
                packed = rpool.tile([128, QPAD * DL // 128], F32)  # [128, 5120]
                nc.vector.tensor_copy(
                    out=packed[:].rearrange("p (r s) -> p r s", s=DL),
                    in_=acc[:].rearrange("p (r s) -> p r s", s=STRIDE)[:, :, :DL])
                nc.sync.dma_start(out=bounce_in.ap(), in_=packed[:])
                nc.gpsimd.collective_compute(
                    "AllReduce", OP.add,
                    replica_groups=[list(range(N_CORES))],
                    ins=[bounce_in.ap().opt()], outs=[bounce_out.ap().opt()])
                nc.sync.dma_start(
                    out=out_d[:, :],
                    in_=bounce_out.ap().rearrange("p f -> (p f)")[
                        :N_ATOMS * DL].rearrange("(a c) -> a c", c=DL))
    nc.compile()
    return nc


def _install_ntff_hook():
    """Provide antenv.axon_hooks (missing in this image) via sys.modules so
    run_bass_kernel_spmd(trace=True) can capture NTFF profiles."""
    import types, ctypes, contextlib
    try:
        from antenv.axon_hooks import get_axon_ntff_profile_hook  # noqa: F401
        return
    except ImportError:
        pass
    so_path = "/opt/axon/libaxon_pjrt.so"
    try:
        lib = ctypes.CDLL(so_path)
    except OSError:
        return
    if not hasattr(lib, "axon_start_nrt_profile"):
        return
    lib.axon_start_nrt_profile.argtypes = [ctypes.POINTER(ctypes.c_int64),
                                           ctypes.c_size_t]
    lib.axon_start_nrt_profile.restype = ctypes.c_int64
    lib.axon_stop_nrt_profile.argtypes = [ctypes.c_char_p]
    lib.axon_stop_nrt_profile.restype = ctypes.c_int64

    @contextlib.contextmanager
    def _hook(output_dir, device_ids):
        import jax
        jax.devices()
        if device_ids:
            ids = (ctypes.c_int64 * len(device_ids))(*device_ids)
            rc = lib.axon_start_nrt_profile(ids, len(device_ids))
        else:
            rc = lib.axon_start_nrt_profile(None, 0)
        if rc != 0:
            raise RuntimeError(f"axon_start_nrt_profile rc={rc}")
        try:
            yield
        finally:
            n = lib.axon_stop_nrt_profile(str(output_dir).encode())
            if n <= 0:
                print(f"ntff capture wrote {n} files", flush=True)

    mod = types.ModuleType("antenv.axon_hooks")
    mod.get_axon_ntff_profile_hook = lambda: _hook
    mod.set_axon_ntff_profile_hook = lambda h: None
    import antenv
    sys.modules["antenv.axon_hooks"] = mod
    antenv.axon_hooks = mod



_CACHE = {}


def kernel(n_atoms, triplet_index, r_ij, r_ik, cos_theta,
           type_i, type_j, type_k, c_table, _sim=False, _trace=False):
    cores, consts, chunks, nmacro, TPAD = _host_prep(
        n_atoms, triplet_index, r_ij, r_ik, cos_theta,
        type_i, type_j, type_k, c_table)
    key = (nmacro, TPAD, tuple(chunks))
    if key not in _CACHE:
        _CACHE[key] = _build(chunks, nmacro, TPAD)
    nc = _CACHE[key]
    in_maps = []
    for c in range(N_CORES):
        m = dict(cores[c])
        m.update(consts)
        in_maps.append(m)
    if _sim:
        from concourse import bass_interp
        sim = bass_interp.MultiCoreSim(nc, N_CORES)
        for c in range(N_CORES):
            for k, v in in_maps[c].items():
                sim.cores[c].tensor(k)[:] = v
        sim.simulate()
        out = np.array(sim.cores[0].mem_tensor("out"))
    else:
        if _trace:
            _install_ntff_hook()
        last_err = None
        for _try in range(3):
            try:
                res = run_bass_kernel_spmd(nc, in_maps,
                                           core_ids=list(range(N_CORES)),
                                           trace=_trace)
                out = np.asarray(res.results[0]["out"])
                break
            except Exception as e:  # transient device-unrecoverable after a crash
                last_err = e
        else:
            raise last_err
        kernel.last_exec_ns = res.exec_time_ns
        kernel.last_results = res
    return out.reshape(N_ATOMS, N_DESC, L_MAX).astype(np.float32)


# revision 43
# speedup vs baseline: 1.3962x; 1.0119x over previous
"""AngularDescriptor Trainium2 kernel (8 NeuronCores, SPMD + AllReduce).

Per core: T/8 triplets.  Device computes Chebyshev/Legendre bases, the
per-pair-type radial einsum (PE matmul with fixed block-diag weights after a
4-way tj/tk one-hot expansion; 4-way ti select on DVE), the outer product
ang = (g_ij*g_ik) (x) P_l, and segment-sums ang into q[20000,8,4] via
gpsimd.dma_scatter_add.  HW scatter-add loses duplicate indices within one
instruction (last-write-wins race), so the host orders each shard's triplets
into occurrence-rank classes (class r = r-th occurrence of an atom): within a
class all atom indices are unique.  Classes are cut into chunks that rotate
over K DRAM accumulators, so same-accumulator scatters serialize (WAW dep)
while different-accumulator scatters overlap.  Padding slots scatter to
distinct dummy atom rows (20000..20479) so one uniform program serves all
cores.  Final: on-device K-way add, AllReduce over the 8 cores, output from
core 0.

v2 layout notes:
 - Chebyshev T_k computed on the Activation engine: with s=(r/rc-1)^2,
   theta = arccos(2s-1) = 2*atan(sqrt(1/s-1)), T_k = sin(pi/2 - 2k*atan(.)).
   The "+1" of the reference basis is folded into the weight table.
 - F1 features are feature-major [p, half, q, k, j] and built with one fused
   DVE op per half; PE transposes merge 2 j-columns per instruction, with the
   weight matrix rows permuted to the (feat,jparity)-interleaved order.
 - A block's SEG=4 triplets occupy 4 consecutive columns of one partition, so
   the per-block pre-reduction is 3 DVE adds (no PE matmuls).
"""
import sys

sys.path.insert(0, "/opt/trn_rl_repo")
import numpy as np

from concourse import bass, bacc, mybir, tile
from concourse.bass_utils import run_bass_kernel_spmd

N_TYPES, N_DESC, K_MAX, L_MAX = 4, 8, 8, 4
R_C = 5.0
N_ATOMS = 20000
N_CORES = 8
DL = N_DESC * L_MAX          # 32
QPAD = 20480                 # 128 * 160
STRIDE = 64                  # q row stride in f32 (256B; scatter needs 256B mult)
J = 128                      # field columns per macro-tile
MACRO = 128 * J              # 16384 triplets per macro
KACC = 2                     # rotating DRAM accumulators
CHUNK = 4096                 # max idxs (blocks) per scatter instruction
SEG = 4                      # triplets pre-reduced per scattered block
F32, BF16, I16 = mybir.dt.float32, mybir.dt.bfloat16, mybir.dt.int16
PI = float(np.pi)
CHEB_ATAN = False            # activation-engine Chebyshev (HW Sin/Arctan input
                             # ranges are too narrow; use the DVE recurrence)


def _host_prep(n_atoms, triplet_index, r_ij, r_ik, cos_theta,
               type_i, type_j, type_k, c_table):
    """Shard; group each atom's triplets into SEG-slot blocks (pad slots use
    r=r_c so fc=0 => ang=0); order blocks by block-rank classes; pad to a
    uniform layout across cores.  Block b's slots live at partition b%128,
    columns SEG*(b//128)..+SEG-1 (block-major columns)."""
    T = triplet_index.shape[0]
    Tc = T // N_CORES
    atom_all = np.asarray(triplet_index[:, 0], dtype=np.int64)

    per_core = []
    max_nblk = 0
    for c in range(N_CORES):
        sl = slice(c * Tc, (c + 1) * Tc)
        atom = atom_all[sl]
        order = np.argsort(atom, kind="stable")
        sa = atom[order]
        first = np.r_[True, sa[1:] != sa[:-1]]
        idxf = np.where(first)[0]
        counts = np.diff(np.r_[idxf, Tc])
        uatoms = sa[idxf]
        nblk_per_atom = -(-counts // SEG)
        per_core.append((sl, atom, order, uatoms, counts, nblk_per_atom))
        max_nblk = max(max_nblk, int(nblk_per_atom.max()))

    nclass = max_nblk
    # per class r: number of blocks (padded to 128) -- uniform across cores
    cls_blk = []
    for r in range(nclass):
        mx = max(int(np.sum(nb > r)) for (_, _, _, _, _, nb) in per_core)
        cls_blk.append(-(-max(mx, 1) // 128) * 128)
    total_blk = sum(cls_blk)
    MB = MACRO // SEG                     # blocks per macro
    nmacro = -(-total_blk // MB)
    TBLK = nmacro * MB
    TPAD = TBLK * SEG

    # chunk table in blocks: (start_blk, len_blk); aligned to macros
    chunks = []
    o = 0
    for r in range(nclass):
        p = cls_blk[r]
        sblk = 0
        while sblk < p:
            cl = min(CHUNK, p - sblk)
            mstart = (o + sblk) // MB
            if (o + sblk + cl - 1) // MB != mstart:
                cl = (mstart + 1) * MB - (o + sblk)
            chunks.append((o + sblk, cl))
            sblk += cl
        o += p
    tail = o
    while tail < TBLK:
        cl = min(CHUNK, TBLK - tail, ((tail // MB) + 1) * MB - tail)
        chunks.append((tail, cl))
        tail += cl

    G = TPAD // 128
    cores = []
    for c in range(N_CORES):
        sl, atom, order, uatoms, counts, nblk = per_core[c]
        f32 = lambda x: np.asarray(x[sl], dtype=np.float32)
        fields = dict(r_ij=f32(r_ij), r_ik=f32(r_ik), ct=f32(cos_theta),
                      ti=f32(type_i), tj=f32(type_j), tk=f32(type_k))
        # pad slots: r=R_C -> fc=0 -> ang=0 exactly; type fields are small
        # ints, exact in bf16 (halves their DMA)
        import ml_dtypes
        dev = {}
        for n in fields:
            fillv = R_C if n in ("r_ij", "r_ik") else 0.0
            dt = ml_dtypes.bfloat16 if n in ("ti", "tj", "tk") else np.float32
            dev[n] = np.full((128, G), fillv, dtype=dt)
        bidx = np.empty(TBLK, dtype=np.int16)
        bidx[:] = (20000 + (np.arange(TBLK, dtype=np.int64) % 480)).astype(np.int16)

        # block table: for each unique atom, blocks 0..nblk-1; block b holds
        # triplets order[start + b*SEG : ...] (clipped); class-major placement
        starts = np.r_[0, np.cumsum(counts)[:-1]]
        o = 0
        for r in range(nclass):
            members = np.where(nblk > r)[0]         # atoms having block r
            if len(members):
                a_st = starts[members] + r * SEG
                a_ct = np.minimum(counts[members] - r * SEG, SEG)
                blk = o + np.arange(len(members))   # destination block ids
                src_idx = []
                dst_p = []
                dst_c = []
                for st, ctn, b in zip(a_st, a_ct, blk):
                    src_idx.append(order[st:st + ctn])
                    dst_p.append(np.full(ctn, b % 128))
                    dst_c.append(SEG * (b // 128) + np.arange(ctn))
                src_idx = np.concatenate(src_idx)
                dst_p = np.concatenate(dst_p)
                dst_c = np.concatenate(dst_c)
                for n in dev:
                    dev[n][dst_p, dst_c] = fields[n][src_idx]
                bidx[o:o + len(members)] = uatoms[members].astype(np.int16)
            o += cls_blk[r]
        arrays = {n: dev[n] for n in dev}
        # idx wrapped [16, TBLK/16] and replicated over the 8 gpsimd cores
        arrays["idx"] = np.tile(bidx.reshape(TBLK // 16, 16).T, (8, 1)).copy()
        cores.append(arrays)

    # weight table with the (T_k + 1) -> T_k fold:
    #   sum_k c[d,k]*(T_k+1)*u = sum_k c'[d,k]*T_k*u,  c'[d,0] += sum_k c[d,k]
    ctab = np.asarray(c_table, dtype=np.float64).copy()
    ctab[:, :, :, 0] += ctab.sum(axis=3)
    ctab = ctab.astype(np.float32)
    W = np.zeros((32, 32), dtype=np.float32)
    for tjv in range(4):
        for k in range(8):
            W[tjv * 8 + k, :] = ctab[:, tjv, :, k].reshape(32)
    W4 = np.zeros((128, 128), dtype=np.float32)
    for b in range(4):
        W4[b * 32:(b + 1) * 32, b * 32:(b + 1) * 32] = W
    # permute rows to the (feat, jparity)-interleaved transpose output order:
    # new row (2f+jj) = old row (jj*64+f)
    W4p = np.zeros_like(W4)
    for jj in range(2):
        for f in range(64):
            W4p[2 * f + jj, :] = W4[jj * 64 + f, :]
    consts = dict(w4=W4p, ident=np.eye(128, dtype=np.float32),
                  iota4=np.tile(np.arange(4, dtype=np.float32), (128, 1)))
    return cores, consts, chunks, nmacro, TPAD


def _build(chunks, nmacro, TPAD, debug=False):
    G = TPAD // 128
    MB = MACRO // SEG
    nc = bacc.Bacc(None, target_bir_lowering=False, num_devices=N_CORES,
                   dynamic_dma_scratch_size=32768, num_swdge_queues=2)
    dbg = {}
    if debug:
        dbg["f1"] = nc.declare_dram_parameter("dbg_f1", [128, 64 * J], F32,
                                              isOutput=True)
        dbg["prodc"] = nc.declare_dram_parameter("dbg_prodc", [128, J * 32], F32,
                                                 isOutput=True)
        dbg["partials"] = nc.declare_dram_parameter("dbg_partials", [128, 32 * 32],
                                                    F32, isOutput=True)
    P = {}
    for n in ("r_ij", "r_ik", "ct", "ti", "tj", "tk"):
        fdt = BF16 if n in ("ti", "tj", "tk") else F32
        P[n] = nc.declare_dram_parameter(n, [128, G], fdt, isOutput=False)
    P["idx"] = nc.declare_dram_parameter("idx", [128, TPAD // SEG // 16], I16,
                                         isOutput=False)
    P["w4"] = nc.declare_dram_parameter("w4", [128, 128], F32, isOutput=False)
    P["ident"] = nc.declare_dram_parameter("ident", [128, 128], F32, isOutput=False)
    P["iota4"] = nc.declare_dram_parameter("iota4", [128, 4], F32, isOutput=False)
    out_d = nc.declare_dram_parameter("out", [N_ATOMS, DL], F32, isOutput=True)

    qacc = [nc.dram_tensor(f"qacc{k}", [QPAD, STRIDE], F32) for k in range(KACC)]
    bounce_in = nc.dram_tensor("bounce_in", [128, QPAD * DL // 128], BF16)
    bounce_out = nc.dram_tensor("bounce_out", [128, QPAD * DL // 128], BF16,
                                addr_space="Shared")

    AF = mybir.ActivationFunctionType
    OP = mybir.AluOpType

    with tile.TileContext(nc) as tc:
        with tc.tile_pool(name="const", bufs=1) as cst:
            w4 = cst.tile([128, 128], BF16)
            ident = cst.tile([128, 128], BF16)
            iota4 = cst.tile([128, 4], F32)
            tmpf = cst.tile([128, 128], F32)
            zero = cst.tile([128, 2048], F32)
            halfpi = cst.tile([128, 1], F32)
            nc.vector.memset(halfpi[:], PI / 2)
            negone = cst.tile([128, 1], F32)
            nc.vector.memset(negone[:], -1.0)
            dbg_t = cst.tile([128, 1024], F32, name="dbg_t") if debug else None
            nc.sync.dma_start(out=tmpf[:], in_=P["w4"][:])
            nc.vector.tensor_copy(out=w4[:], in_=tmpf[:])
            nc.sync.dma_start(out=tmpf[:], in_=P["ident"][:])
            nc.vector.tensor_copy(out=ident[:], in_=tmpf[:])
            nc.sync.dma_start(out=iota4[:], in_=P["iota4"][:])
            nc.vector.memset(zero[:], 0.0)
            qf0 = qacc[0].ap().rearrange("(p r) s -> p (r s)", p=128)
            w0 = QPAD * STRIDE // 128
            for i in range(0, w0, 2048):
                nc.sync.dma_start(out=qf0[:, i:i + 2048],
                                  in_=zero[:, :min(2048, w0 - i)])

            with (
                tc.tile_pool(name="fields", bufs=3) as fpool,
                tc.tile_pool(name="idxp", bufs=3) as ipool,
                tc.tile_pool(name="work", bufs=2) as wpool,
                tc.tile_pool(name="f1p", bufs=3) as f1pool,
                tc.tile_pool(name="prp", bufs=3) as prpool,
                tc.tile_pool(name="cinp", bufs=4) as cpool,
                tc.tile_pool(name="cndp", bufs=2) as cndpool,
                tc.tile_pool(name="angp", bufs=2) as apool,
                tc.tile_pool(name="scat", bufs=3) as spool,
                tc.tile_pool(name="ps1", bufs=3, space="PSUM") as ppool1,
                tc.tile_pool(name="ps2", bufs=3, space="PSUM") as ppool2,
            ):
                by_macro = [[] for _ in range(nmacro)]
                for ci, (s, pl) in enumerate(chunks):
                    by_macro[s // MB].append((ci, s % MB, pl))
                NIC = MB // 16   # idx cols per macro

                def issue_loads(m):
                    """Field/idx DMAs + the u/s activations for macro m.
                    Called one iteration ahead so the scalar-queue Sin/Square
                    land before the previous macro's cin/cnd copies."""
                    fld = {}
                    for n in ("r_ij", "r_ik", "ct", "ti", "tj", "tk"):
                        fdt = BF16 if n in ("ti", "tj", "tk") else F32
                        t = fpool.tile([128, J], fdt, name=f"fld_{n}")
                        nc.sync.dma_start(out=t[:], in_=P[n][:, m * J:(m + 1) * J])
                        fld[n] = t
                    idxs = ipool.tile([128, NIC], I16, name="idxs")
                    nc.sync.dma_start(out=idxs[:],
                                      in_=P["idx"][:, m * NIC:(m + 1) * NIC])
                    us = []
                    for half, rn in enumerate(("r_ij", "r_ik")):
                        r = fld[rn]
                        u = wpool.tile([128, J], F32, name=f"u{half}")
                        nc.scalar.activation(u[:], r[:], AF.Sin,
                                             bias=halfpi[:], scale=-PI / R_C)
                        s = wpool.tile([128, J], F32, name=f"s{half}")
                        nc.scalar.activation(s[:], r[:], AF.Square,
                                             bias=negone[:], scale=1.0 / R_C)
                        us.append((u, s))
                    return fld, idxs, us

                pre = issue_loads(0)
                for k in range(1, KACC):
                    # qacc1 zeroing rides behind macro-0 loads; its first
                    # scatter is a macro later (qacc0 was zeroed upfront)
                    qf = qacc[k].ap().rearrange("(p r) s -> p (r s)", p=128)
                    w = QPAD * STRIDE // 128
                    for i in range(0, w, 2048):
                        nc.sync.dma_start(out=qf[:, i:i + 2048],
                                          in_=zero[:, :min(2048, w - i)])

                for m in range(nmacro):
                    fld, idxs, us = pre
                    if m + 1 < nmacro:
                        pre = issue_loads(m + 1)

                    # ---- bases + one-hot expanded features ----
                    # layout [p, jpair, feat=(h,q,k), jparity]: a j-pair's 128
                    # features are contiguous -> one full-width PE transpose
                    F1 = f1pool.tile([128, J // 2, 64, 2], BF16)
                    Fv = F1[:].rearrange("p jp (h q k) jj -> p h jp q k jj",
                                         h=2, q=4)
                    hx = []
                    for half in range(2):
                        u, s = us[half]
                        nc.vector.tensor_scalar(out=u[:], in0=u[:], scalar1=0.25,
                                                scalar2=0.25, op0=OP.mult,
                                                op1=OP.add)
                        x = wpool.tile([128, J], F32, name=f"x{half}")
                        nc.vector.tensor_scalar(out=x[:], in0=s[:], scalar1=2.0,
                                                scalar2=-1.0, op0=OP.mult,
                                                op1=OP.add)
                        x2 = wpool.tile([128, J], F32, name=f"x2{half}")
                        nc.vector.tensor_scalar_mul(x2[:], x[:], 2.0)
                        Traw = wpool.tile([128, 8, J], F32, name=f"Traw{half}")
                        nc.vector.memset(Traw[:, 0, :], 1.0)
                        nc.vector.tensor_copy(out=Traw[:, 1, :], in_=x[:])
                        hx.append((u, x2, Traw))
                    # interleaved recurrence: the two chains hide each other's
                    # SBUF write-ack latency
                    for k in range(2, 8):
                        for half in range(2):
                            u, x2, Traw = hx[half]
                            nc.vector.tensor_tensor(out=Traw[:, k, :], in0=x2[:],
                                                    in1=Traw[:, k - 1, :],
                                                    op=OP.mult)
                            nc.vector.tensor_tensor(out=Traw[:, k, :],
                                                    in0=Traw[:, k, :],
                                                    in1=Traw[:, k - 2, :],
                                                    op=OP.subtract)
                    for half, tn in enumerate(("tj", "tk")):
                        u, x2, Traw = hx[half]
                        # Tu[jp, k, jj] = T_k(2jp+jj) * u(2jp+jj)  (pair-contig)
                        Tu = wpool.tile([128, J // 2, 8, 2], BF16,
                                        name=f"Tu{half}")
                        Trs = Traw[:].rearrange("p k (jp jj) -> p jp k jj", jj=2)
                        ubs = u[:].rearrange("p (jp jj) -> p jp jj", jj=2)
                        nc.vector.tensor_tensor(
                            out=Tu[:], in0=Trs,
                            in1=ubs.unsqueeze(2).broadcast_to([128, J // 2, 8, 2]),
                            op=OP.mult)
                        # oh[jp, q, jj] = (tq(2jp+jj) == q)  (pair-contig)
                        oh = wpool.tile([128, J // 2, 4, 2], BF16,
                                        name=f"oh{half}")
                        tq = fld[tn]
                        tqs = tq[:].rearrange("p (jp jj) -> p jp jj", jj=2)
                        nc.vector.tensor_tensor(
                            out=oh[:],
                            in0=tqs.unsqueeze(2).broadcast_to([128, J // 2, 4, 2]),
                            in1=iota4[:].unsqueeze(1).unsqueeze(3)
                                .broadcast_to([128, J // 2, 4, 2]),
                            op=OP.is_equal)
                        # F1[jp, (half,q,k), jj] = Tu[jp, k, jj] * oh[jp, q, jj]
                        for jj in range(2):
                            nc.vector.tensor_tensor(
                                out=Fv[:, half, :, :, :, jj],
                                in0=Tu[:, :, :, jj].unsqueeze(2)
                                    .broadcast_to([128, J // 2, 4, 8]),
                                in1=oh[:, :, :, jj].unsqueeze(3)
                                    .broadcast_to([128, J // 2, 4, 8]),
                                op=OP.mult)

                    oh_ti = wpool.tile([128, J, 4], BF16)
                    nc.vector.tensor_tensor(
                        out=oh_ti[:],
                        in0=fld["ti"][:].unsqueeze(2).broadcast_to([128, J, 4]),
                        in1=iota4[:].unsqueeze(1).broadcast_to([128, J, 4]),
                        op=OP.is_equal)

                    # Legendre (bf16): P0=1, P1=ct, P2=1.5ct^2-.5, P3=ct(2.5ct^2-1.5)
                    ctf = fld["ct"]
                    P4 = wpool.tile([128, J, 4], BF16)
                    ct2 = wpool.tile([128, J], F32)
                    nc.vector.tensor_tensor(out=ct2[:], in0=ctf[:], in1=ctf[:],
                                            op=OP.mult)
                    nc.vector.memset(P4[:, :, 0], 1.0)
                    nc.vector.tensor_copy(out=P4[:, :, 1], in_=ctf[:])
                    nc.vector.tensor_scalar(out=P4[:, :, 2], in0=ct2[:], scalar1=1.5,
                                            scalar2=-0.5, op0=OP.mult, op1=OP.add)
                    p3t = wpool.tile([128, J], F32)
                    nc.vector.tensor_scalar(out=p3t[:], in0=ct2[:], scalar1=2.5,
                                            scalar2=-1.5, op0=OP.mult, op1=OP.add)
                    nc.vector.tensor_tensor(out=p3t[:], in0=p3t[:], in1=ctf[:],
                                            op=OP.mult)
                    nc.vector.tensor_copy(out=P4[:, :, 3], in_=p3t[:])

                    # ---- PE: per 8-column group; product once per 4 groups ----
                    prodc = prpool.tile([128, J, 32], BF16)
                    F1m = F1[:].rearrange("p jp f jj -> p jp (f jj)")
                    NG = J // 8
                    for g4 in range(NG // 4):
                        cndb = cndpool.tile([128, 4, 4, 128], BF16, name="cndb")
                        for gs in range(4):
                            grp = g4 * 4 + gs
                            ps1 = ppool1.tile([128, 4, 128], BF16, space="PSUM",
                                              name="ps1")
                            for c in range(4):
                                nc.tensor.transpose(
                                    out=ps1[:, c, :], in_=F1m[:, 4 * grp + c, :],
                                    identity=ident[:])
                            cin = cpool.tile([128, 4, 128], BF16, name="cin")
                            nc.scalar.activation(
                                cin[:].rearrange("p c f -> p (c f)"),
                                ps1[:].rearrange("p c f -> p (c f)"), AF.Identity)
                            ps2 = ppool2.tile([128, 4, 128], F32, space="PSUM",
                                              name="ps2")
                            for c in range(4):
                                nc.tensor.matmul(out=ps2[:, c, :], lhsT=cin[:, c, :],
                                                 rhs=w4[:], start=True, stop=True)
                            nc.scalar.activation(
                                cndb[:, gs, :, :].rearrange("p c f -> p (c f)"),
                                ps2[:].rearrange("p c f -> p (c f)"), AF.Identity)
                        # prodc[8(4g4+gs)+2c+jj, f] = g_ij * g_ik
                        psv = cndb[:].rearrange("p g c (j h f) -> p (g c) j h f",
                                                j=2, h=2)
                        nc.vector.tensor_tensor(
                            out=prodc[:, g4 * 32:(g4 + 1) * 32, :]
                                .rearrange("p (gc j) f -> p gc j f", j=2),
                            in0=psv[:, :, :, 0, :], in1=psv[:, :, :, 1, :],
                            op=OP.mult)

                    # ---- ti select: h = sum_q ohti[q] * prodc[:, :, q*8:(q+1)*8]
                    h = wpool.tile([128, J, 8], BF16)
                    t0 = wpool.tile([128, J, 8], BF16)
                    nc.vector.tensor_tensor(
                        out=h[:], in0=prodc[:, :, 0:8],
                        in1=oh_ti[:, :, 0].unsqueeze(2).broadcast_to([128, J, 8]),
                        op=OP.mult)
                    for q in range(1, 4):
                        nc.vector.tensor_tensor(
                            out=t0[:], in0=prodc[:, :, q * 8:(q + 1) * 8],
                            in1=oh_ti[:, :, q].unsqueeze(2).broadcast_to([128, J, 8]),
                            op=OP.mult)
                        nc.vector.tensor_tensor(out=h[:], in0=h[:], in1=t0[:],
                                                op=OP.add)

                    # ---- ang = h (x) P (bf16) ----
                    ang = apool.tile([128, J, 8, 4], BF16)
                    nc.vector.tensor_tensor(
                        out=ang[:],
                        in0=h[:].unsqueeze(3).broadcast_to([128, J, 8, 4]),
                        in1=P4[:].unsqueeze(2).broadcast_to([128, J, 8, 4]),
                        op=OP.mult)

                    # ---- per-block pre-reduction: 3 adds over 4 columns ----
                    partials = spool.tile([128, J // SEG, 32], F32)
                    av = ang[:].rearrange("p (g s) d l -> p g s (d l)", s=SEG)
                    ts1 = wpool.tile([128, J // SEG, 32], BF16)
                    ts2 = wpool.tile([128, J // SEG, 32], BF16)
                    nc.vector.tensor_tensor(out=ts1[:], in0=av[:, :, 0, :],
                                            in1=av[:, :, 1, :], op=OP.add)
                    nc.vector.tensor_tensor(out=ts2[:], in0=av[:, :, 2, :],
                                            in1=av[:, :, 3, :], op=OP.add)
                    nc.vector.tensor_tensor(out=partials[:], in0=ts1[:],
                                            in1=ts2[:], op=OP.add)

                    if debug and m == 0:
                        for nm, src, w in (
                            ("f1", F1[:].rearrange("p jp f jj -> p (jp f jj)"), 64 * J),
                            ("prodc", prodc[:].rearrange("p j f -> p (j f)"), J * 32),
                            ("partials", partials[:].rearrange("p g f -> p (g f)"),
                             32 * 32),
                        ):
                            for i in range(0, w, 1024):
                                e = min(i + 1024, w)
                                nc.vector.tensor_copy(out=dbg_t[:, :e - i],
                                                      in_=src[:, i:e])
                                nc.sync.dma_start(out=dbg[nm].ap()[:, i:e],
                                                  in_=dbg_t[:, :e - i])

                    # ---- scatter chunks of this macro (block granularity) ----
                    for (ci, off, pl) in by_macro[m]:
                        nc.gpsimd.dma_scatter_add(
                            qacc[ci % KACC].ap()[:, :DL],
                            partials[:, off // 128:(off + pl) // 128, :],
                            idxs[:, off // 16:(off + pl) // 16],
                            pl, pl, DL, elem_step=STRIDE,
                            queue_num=ci % 2)

            # ---- reduce K accumulators, AllReduce, emit ----
            with tc.tile_pool(name="red", bufs=1) as rpool:
                W = QPAD * STRIDE // 128   # 10240
                acc = rpool.tile([128, W], F32)
                tmp = rpool.tile([128, W], F32)
                qv = [q.ap().rearrange("(p r) s -> p (r s)", p=128) for q in qacc]
                nc.sync.dma_start(out=acc[:], in_=qv[0])
                nc.sync.dma_start(out=tmp[:], in_=qv[1])
                nc.vector.tensor_tensor(out=acc[:], in0=acc[:], in1=tmp[:], op=OP.add)
                packed = rpool.tile([128, QPAD * DL // 128], BF16)  # [128, 5120]
                nc.vector.tensor_copy(
                    out=packed[:].rearrange("p (r s) -> p r s", s=DL),
                    in_=acc[:].rearrange("p (r s) -> p r s", s=STRIDE)[:, :, :DL])
                nc.sync.dma_start(out=bounce_in.ap(), in_=packed[:])
                nc.gpsimd.collective_compute(
                    "AllReduce", OP.add,
                    replica_groups=[list(range(N_CORES))],
                    ins=[bounce_in.ap().opt()], outs=[bounce_out.ap().opt()])
                obf = rpool.tile([128, QPAD * DL // 128], BF16)
                nc.sync.dma_start(out=obf[:], in_=bounce_out.ap())
                of32 = rpool.tile([128, QPAD * DL // 128], F32)
                nc.vector.tensor_copy(out=of32[:], in_=obf[:])
                nc.sync.dma_start(
                    out=out_d.ap().rearrange("(p r) c -> p (r c)", r=QPAD // 128),
                    in_=of32[:N_ATOMS * 128 // QPAD, :])
    nc.compile()
    return nc


def _install_ntff_hook():
    """Provide antenv.axon_hooks (missing in this image) via sys.modules so
    run_bass_kernel_spmd(trace=True) can capture NTFF profiles."""
    import types, ctypes, contextlib
    try:
        from antenv.axon_hooks import get_axon_ntff_profile_hook  # noqa: F401
        return
    except ImportError:
        pass
    so_path = "/opt/axon/libaxon_pjrt.so"
    try:
        lib = ctypes.CDLL(so_path)
    except OSError:
        return
    if not hasattr(lib, "axon_start_nrt_profile"):
        return
    lib.axon_start_nrt_profile.argtypes = [ctypes.POINTER(ctypes.c_int64),
                                           ctypes.c_size_t]
    lib.axon_start_nrt_profile.restype = ctypes.c_int64
    lib.axon_stop_nrt_profile.argtypes = [ctypes.c_char_p]
    lib.axon_stop_nrt_profile.restype = ctypes.c_int64

    @contextlib.contextmanager
    def _hook(output_dir, device_ids):
        import jax
        jax.devices()
        if device_ids:
            ids = (ctypes.c_int64 * len(device_ids))(*device_ids)
            rc = lib.axon_start_nrt_profile(ids, len(device_ids))
        else:
            rc = lib.axon_start_nrt_profile(None, 0)
        if rc != 0:
            raise RuntimeError(f"axon_start_nrt_profile rc={rc}")
        try:
            yield
        finally:
            n = lib.axon_stop_nrt_profile(str(output_dir).encode())
            if n <= 0:
                print(f"ntff capture wrote {n} files", flush=True)

    mod = types.ModuleType("antenv.axon_hooks")
    mod.get_axon_ntff_profile_hook = lambda: _hook
    mod.set_axon_ntff_profile_hook = lambda h: None
    import antenv
    sys.modules["antenv.axon_hooks"] = mod
    antenv.axon_hooks = mod



_CACHE = {}


def kernel(n_atoms, triplet_index, r_ij, r_ik, cos_theta,
           type_i, type_j, type_k, c_table, _sim=False, _trace=False):
    cores, consts, chunks, nmacro, TPAD = _host_prep(
        n_atoms, triplet_index, r_ij, r_ik, cos_theta,
        type_i, type_j, type_k, c_table)
    key = (nmacro, TPAD, tuple(chunks))
    if key not in _CACHE:
        _CACHE[key] = _build(chunks, nmacro, TPAD)
    nc = _CACHE[key]
    in_maps = []
    for c in range(N_CORES):
        m = dict(cores[c])
        m.update(consts)
        in_maps.append(m)
    if _sim:
        from concourse import bass_interp
        sim = bass_interp.MultiCoreSim(nc, N_CORES)
        for c in range(N_CORES):
            for k, v in in_maps[c].items():
                sim.cores[c].tensor(k)[:] = v
        sim.simulate()
        out = np.array(sim.cores[0].mem_tensor("out"))
    else:
        if _trace:
            _install_ntff_hook()
        last_err = None
        for _try in range(3):
            try:
                res = run_bass_kernel_spmd(nc, in_maps,
                                           core_ids=list(range(N_CORES)),
                                           trace=_trace)
                out = np.asarray(res.results[0]["out"])
                break
            except Exception as e:  # transient device-unrecoverable after a crash
                last_err = e
        else:
            raise last_err
        kernel.last_exec_ns = res.exec_time_ns
        kernel.last_results = res
    return out.reshape(N_ATOMS, N_DESC, L_MAX).astype(np.float32)


# revision 46
# speedup vs baseline: 1.8262x; 1.3080x over previous
"""AngularDescriptor Trainium2 kernel (8 NeuronCores, SPMD + AllReduce).

Per core: T/8 triplets.  Device computes Chebyshev/Legendre bases, the
per-pair-type radial einsum (PE matmul with fixed block-diag weights after a
4-way tj/tk one-hot expansion; 4-way ti select on DVE), the outer product
ang = (g_ij*g_ik) (x) P_l, and segment-sums ang into q[20000,8,4] via
gpsimd.dma_scatter_add.  HW scatter-add loses duplicate indices within one
instruction (last-write-wins race), so the host orders each shard's triplets
into occurrence-rank classes (class r = r-th occurrence of an atom): within a
class all atom indices are unique.  Classes are cut into chunks that rotate
over K DRAM accumulators, so same-accumulator scatters serialize (WAW dep)
while different-accumulator scatters overlap.  Padding slots scatter to
distinct dummy atom rows (20000..20479) so one uniform program serves all
cores.  Final: on-device K-way add, AllReduce over the 8 cores, output from
core 0.

v2 layout notes:
 - Chebyshev T_k computed on the Activation engine: with s=(r/rc-1)^2,
   theta = arccos(2s-1) = 2*atan(sqrt(1/s-1)), T_k = sin(pi/2 - 2k*atan(.)).
   The "+1" of the reference basis is folded into the weight table.
 - F1 features are feature-major [p, half, q, k, j] and built with one fused
   DVE op per half; PE transposes merge 2 j-columns per instruction, with the
   weight matrix rows permuted to the (feat,jparity)-interleaved order.
 - A block's SEG=4 triplets occupy 4 consecutive columns of one partition, so
   the per-block pre-reduction is 3 DVE adds (no PE matmuls).
"""
import sys

sys.path.insert(0, "/opt/trn_rl_repo")
import numpy as np

from concourse import bass, bacc, mybir, tile
from concourse.bass_utils import run_bass_kernel_spmd

N_TYPES, N_DESC, K_MAX, L_MAX = 4, 8, 8, 4
R_C = 5.0
N_ATOMS = 20000
N_CORES = 8
DL = N_DESC * L_MAX          # 32
QPAD = 20480                 # 128 * 160
STRIDE = 64                  # q row stride in f32 (256B; scatter needs 256B mult)
J = 128                      # field columns per macro-tile
MACRO = 128 * J              # 16384 triplets per macro
KACC = 2                     # rotating DRAM accumulators
CHUNK = 4096                 # max idxs (blocks) per scatter instruction
SEG = 4                      # triplets pre-reduced per scattered block
F32, BF16, I16 = mybir.dt.float32, mybir.dt.bfloat16, mybir.dt.int16
PI = float(np.pi)
CHEB_ATAN = False            # activation-engine Chebyshev (HW Sin/Arctan input
                             # ranges are too narrow; use the DVE recurrence)


def _host_prep(n_atoms, triplet_index, r_ij, r_ik, cos_theta,
               type_i, type_j, type_k, c_table):
    """Shard; group each atom's triplets into SEG-slot blocks (pad slots use
    r=r_c so fc=0 => ang=0); order blocks by block-rank classes; pad to a
    uniform layout across cores.  Block b's slots live at partition b%128,
    columns SEG*(b//128)..+SEG-1 (block-major columns)."""
    T = triplet_index.shape[0]
    Tc = T // N_CORES
    atom_all = np.asarray(triplet_index[:, 0], dtype=np.int64)

    per_core = []
    max_nblk = 0
    for c in range(N_CORES):
        sl = slice(c * Tc, (c + 1) * Tc)
        atom = atom_all[sl]
        order = np.argsort(atom, kind="stable")
        sa = atom[order]
        first = np.r_[True, sa[1:] != sa[:-1]]
        idxf = np.where(first)[0]
        counts = np.diff(np.r_[idxf, Tc])
        uatoms = sa[idxf]
        nblk_per_atom = -(-counts // SEG)
        per_core.append((sl, atom, order, uatoms, counts, nblk_per_atom))
        max_nblk = max(max_nblk, int(nblk_per_atom.max()))

    nclass = max_nblk
    # per class r: number of blocks (padded to 128) -- uniform across cores
    cls_blk = []
    for r in range(nclass):
        mx = max(int(np.sum(nb > r)) for (_, _, _, _, _, nb) in per_core)
        cls_blk.append(-(-max(mx, 1) // 128) * 128)
    total_blk = sum(cls_blk)
    MB = MACRO // SEG                     # blocks per macro
    nmacro = -(-total_blk // MB)
    TBLK = nmacro * MB
    TPAD = TBLK * SEG

    # chunk table in blocks: (start_blk, len_blk); aligned to macros
    chunks = []
    o = 0
    for r in range(nclass):
        p = cls_blk[r]
        sblk = 0
        while sblk < p:
            cl = min(CHUNK, p - sblk)
            mstart = (o + sblk) // MB
            if (o + sblk + cl - 1) // MB != mstart:
                cl = (mstart + 1) * MB - (o + sblk)
            chunks.append((o + sblk, cl))
            sblk += cl
        o += p
    tail = o
    while tail < TBLK:
        cl = min(CHUNK, TBLK - tail, ((tail // MB) + 1) * MB - tail)
        chunks.append((tail, cl))
        tail += cl

    G = TPAD // 128
    cores = []
    for c in range(N_CORES):
        sl, atom, order, uatoms, counts, nblk = per_core[c]
        f32 = lambda x: np.asarray(x[sl], dtype=np.float32)
        fields = dict(r_ij=f32(r_ij), r_ik=f32(r_ik), ct=f32(cos_theta),
                      ti=f32(type_i), tj=f32(type_j), tk=f32(type_k))
        # pad slots: r=R_C -> fc=0 -> ang=0 exactly; type fields are small
        # ints, exact in bf16 (halves their DMA)
        import ml_dtypes
        dev = {}
        for n in fields:
            fillv = R_C if n in ("r_ij", "r_ik") else 0.0
            dt = ml_dtypes.bfloat16 if n in ("ti", "tj", "tk") else np.float32
            dev[n] = np.full((128, G), fillv, dtype=dt)
        bidx = np.empty(TBLK, dtype=np.int16)
        bidx[:] = (20000 + (np.arange(TBLK, dtype=np.int64) % 480)).astype(np.int16)

        # block table: for each unique atom, blocks 0..nblk-1; block b holds
        # triplets order[start + b*SEG : ...] (clipped); class-major placement
        starts = np.r_[0, np.cumsum(counts)[:-1]]
        o = 0
        for r in range(nclass):
            members = np.where(nblk > r)[0]         # atoms having block r
            if len(members):
                a_st = starts[members] + r * SEG
                a_ct = np.minimum(counts[members] - r * SEG, SEG)
                blk = o + np.arange(len(members))   # destination block ids
                src_idx = []
                dst_p = []
                dst_c = []
                for st, ctn, b in zip(a_st, a_ct, blk):
                    src_idx.append(order[st:st + ctn])
                    dst_p.append(np.full(ctn, b % 128))
                    dst_c.append(SEG * (b // 128) + np.arange(ctn))
                src_idx = np.concatenate(src_idx)
                dst_p = np.concatenate(dst_p)
                dst_c = np.concatenate(dst_c)
                for n in dev:
                    dev[n][dst_p, dst_c] = fields[n][src_idx]
                bidx[o:o + len(members)] = uatoms[members].astype(np.int16)
            o += cls_blk[r]
        arrays = {n: dev[n] for n in dev}
        # idx wrapped [16, TBLK/16] and replicated over the 8 gpsimd cores
        arrays["idx"] = np.tile(bidx.reshape(TBLK // 16, 16).T, (8, 1)).copy()
        cores.append(arrays)

    # weight table with the (T_k + 1) -> T_k fold:
    #   sum_k c[d,k]*(T_k+1)*u = sum_k c'[d,k]*T_k*u,  c'[d,0] += sum_k c[d,k]
    ctab = np.asarray(c_table, dtype=np.float64).copy()
    ctab[:, :, :, 0] += ctab.sum(axis=3)
    ctab = ctab.astype(np.float32)
    W = np.zeros((32, 32), dtype=np.float32)
    for tjv in range(4):
        for k in range(8):
            W[tjv * 8 + k, :] = ctab[:, tjv, :, k].reshape(32)
    W4 = np.zeros((128, 128), dtype=np.float32)
    for b in range(4):
        W4[b * 32:(b + 1) * 32, b * 32:(b + 1) * 32] = W
    # permute rows to the (feat, jparity)-interleaved transpose output order:
    # new row (2f+jj) = old row (jj*64+f)
    W4p = np.zeros_like(W4)
    for jj in range(2):
        for f in range(64):
            W4p[2 * f + jj, :] = W4[jj * 64 + f, :]
    consts = dict(w4=W4p, ident=np.eye(128, dtype=np.float32),
                  iota4=np.tile(np.arange(4, dtype=np.float32), (128, 1)))
    return cores, consts, chunks, nmacro, TPAD


def _build(chunks, nmacro, TPAD, debug=False):
    G = TPAD // 128
    MB = MACRO // SEG
    nc = bacc.Bacc(None, target_bir_lowering=False, num_devices=N_CORES,
                   dynamic_dma_scratch_size=32768, num_swdge_queues=2)
    dbg = {}
    if debug:
        dbg["f1"] = nc.declare_dram_parameter("dbg_f1", [128, 64 * J], F32,
                                              isOutput=True)
        dbg["prodc"] = nc.declare_dram_parameter("dbg_prodc", [128, J * 32], F32,
                                                 isOutput=True)
        dbg["partials"] = nc.declare_dram_parameter("dbg_partials", [128, 32 * 32],
                                                    F32, isOutput=True)
    P = {}
    for n in ("r_ij", "r_ik", "ct", "ti", "tj", "tk"):
        fdt = BF16 if n in ("ti", "tj", "tk") else F32
        P[n] = nc.declare_dram_parameter(n, [128, G], fdt, isOutput=False)
    P["idx"] = nc.declare_dram_parameter("idx", [128, TPAD // SEG // 16], I16,
                                         isOutput=False)
    P["w4"] = nc.declare_dram_parameter("w4", [128, 128], F32, isOutput=False)
    P["ident"] = nc.declare_dram_parameter("ident", [128, 128], F32, isOutput=False)
    P["iota4"] = nc.declare_dram_parameter("iota4", [128, 4], F32, isOutput=False)
    out_d = nc.declare_dram_parameter("out", [N_ATOMS, DL], F32, isOutput=True)

    qacc = [nc.dram_tensor(f"qacc{k}", [QPAD, STRIDE], F32) for k in range(KACC)]
    bounce_in = nc.dram_tensor("bounce_in", [128, QPAD * DL // 128], BF16)
    bounce_out = nc.dram_tensor("bounce_out", [128, QPAD * DL // 128], BF16,
                                addr_space="Shared")

    AF = mybir.ActivationFunctionType
    OP = mybir.AluOpType

    with tile.TileContext(nc) as tc:
        with tc.tile_pool(name="const", bufs=1) as cst:
            w4 = cst.tile([128, 128], BF16)
            ident = cst.tile([128, 128], BF16)
            iota4 = cst.tile([128, 4], F32)
            tmpf = cst.tile([128, 128], F32)
            zero = cst.tile([128, 2048], F32)
            halfpi = cst.tile([128, 1], F32)
            nc.vector.memset(halfpi[:], PI / 2)
            negone = cst.tile([128, 1], F32)
            nc.vector.memset(negone[:], -1.0)
            dbg_t = cst.tile([128, 1024], F32, name="dbg_t") if debug else None
            nc.sync.dma_start(out=tmpf[:], in_=P["w4"][:])
            nc.vector.tensor_copy(out=w4[:], in_=tmpf[:])
            nc.sync.dma_start(out=tmpf[:], in_=P["ident"][:])
            nc.vector.tensor_copy(out=ident[:], in_=tmpf[:])
            nc.sync.dma_start(out=iota4[:], in_=P["iota4"][:])
            nc.vector.memset(zero[:], 0.0)
            qf0 = qacc[0].ap().rearrange("(p r) s -> p (r s)", p=128)
            w0 = QPAD * STRIDE // 128
            for i in range(0, w0, 2048):
                nc.sync.dma_start(out=qf0[:, i:i + 2048],
                                  in_=zero[:, :min(2048, w0 - i)])

            with (
                tc.tile_pool(name="fields", bufs=3) as fpool,
                tc.tile_pool(name="idxp", bufs=4) as ipool,
                tc.tile_pool(name="work", bufs=2) as wpool,
                tc.tile_pool(name="f1p", bufs=3) as f1pool,
                tc.tile_pool(name="prp", bufs=3) as prpool,
                tc.tile_pool(name="cinp", bufs=4) as cpool,
                tc.tile_pool(name="cndp", bufs=2) as cndpool,
                tc.tile_pool(name="angp", bufs=2) as apool,
                tc.tile_pool(name="scat", bufs=3) as spool,
                tc.tile_pool(name="ps1", bufs=3, space="PSUM") as ppool1,
                tc.tile_pool(name="ps2", bufs=3, space="PSUM") as ppool2,
            ):
                by_macro = [[] for _ in range(nmacro)]
                for ci, (s, pl) in enumerate(chunks):
                    by_macro[s // MB].append((ci, s % MB, pl))
                NIC = MB // 16   # idx cols per macro

                def issue_loads(m):
                    """Field/idx DMAs + the u/s activations for macro m.
                    Called one iteration ahead so the scalar-queue Sin/Square
                    land before the previous macro's cin/cnd copies."""
                    fld = {}
                    for n in ("r_ij", "r_ik", "ct", "ti", "tj", "tk"):
                        fdt = BF16 if n in ("ti", "tj", "tk") else F32
                        t = fpool.tile([128, J], fdt, name=f"fld_{n}")
                        nc.sync.dma_start(out=t[:], in_=P[n][:, m * J:(m + 1) * J])
                        fld[n] = t
                    idxs = ipool.tile([128, NIC], I16, name="idxs")
                    nc.sync.dma_start(out=idxs[:],
                                      in_=P["idx"][:, m * NIC:(m + 1) * NIC])
                    us = []
                    for half, rn in enumerate(("r_ij", "r_ik")):
                        r = fld[rn]
                        # u = 0.5*cos^2(pi*r/(2*rc)) = 0.25*cos(pi*r/rc)+0.25
                        u = wpool.tile([128, J], F32, name=f"u{half}")
                        nc.scalar.activation(u[:], r[:], AF.Sin,
                                             bias=halfpi[:], scale=-PI / (2 * R_C))
                        nc.scalar.activation(u[:], u[:], AF.Square,
                                             scale=float(np.sqrt(0.5)))
                        s = wpool.tile([128, J], F32, name=f"s{half}")
                        nc.scalar.activation(s[:], r[:], AF.Square,
                                             bias=negone[:], scale=1.0 / R_C)
                        us.append((u, s))
                    return fld, idxs, us

                pre = issue_loads(0)
                for k in range(1, KACC):
                    # qacc1 zeroing rides behind macro-0 loads; its first
                    # scatter is a macro later (qacc0 was zeroed upfront)
                    qf = qacc[k].ap().rearrange("(p r) s -> p (r s)", p=128)
                    w = QPAD * STRIDE // 128
                    for i in range(0, w, 2048):
                        nc.sync.dma_start(out=qf[:, i:i + 2048],
                                          in_=zero[:, :min(2048, w - i)])

                for m in range(nmacro):
                    fld, idxs, us = pre
                    if m + 1 < nmacro:
                        pre = issue_loads(m + 1)

                    # ---- bases + one-hot expanded features ----
                    # layout [p, jpair, feat=(h,q,k), jparity]: a j-pair's 128
                    # features are contiguous -> one full-width PE transpose
                    F1 = f1pool.tile([128, J // 2, 64, 2], BF16)
                    Fv = F1[:].rearrange("p jp (h q k) jj -> p h jp q k jj",
                                         h=2, q=4)
                    hx = []
                    for half in range(2):
                        u, s = us[half]
                        Traw = wpool.tile([128, 8, J], F32, name=f"Traw{half}")
                        nc.vector.tensor_scalar(out=Traw[:, 1, :], in0=s[:],
                                                scalar1=2.0, scalar2=-1.0,
                                                op0=OP.mult, op1=OP.add)
                        x2 = wpool.tile([128, J], F32, name=f"x2{half}")
                        nc.vector.tensor_scalar_mul(x2[:], Traw[:, 1, :], 2.0)
                        nc.vector.memset(Traw[:, 0, :], 1.0)
                        hx.append((u, x2, Traw))
                    # interleaved recurrence: the two chains hide each other's
                    # SBUF write-ack latency
                    for k in range(2, 8):
                        for half in range(2):
                            u, x2, Traw = hx[half]
                            nc.vector.tensor_tensor(out=Traw[:, k, :], in0=x2[:],
                                                    in1=Traw[:, k - 1, :],
                                                    op=OP.mult)
                            nc.vector.tensor_tensor(out=Traw[:, k, :],
                                                    in0=Traw[:, k, :],
                                                    in1=Traw[:, k - 2, :],
                                                    op=OP.subtract)
                    for half, tn in enumerate(("tj", "tk")):
                        u, x2, Traw = hx[half]
                        # Tu[jp, k, jj] = T_k(2jp+jj) * u(2jp+jj)  (pair-contig)
                        Tu = wpool.tile([128, J // 2, 8, 2], BF16,
                                        name=f"Tu{half}")
                        Trs = Traw[:].rearrange("p k (jp jj) -> p jp k jj", jj=2)
                        ubs = u[:].rearrange("p (jp jj) -> p jp jj", jj=2)
                        nc.vector.tensor_tensor(
                            out=Tu[:], in0=Trs,
                            in1=ubs.unsqueeze(2).broadcast_to([128, J // 2, 8, 2]),
                            op=OP.mult)
                        # oh[jp, q, jj] = (tq(2jp+jj) == q)  (pair-contig)
                        oh = wpool.tile([128, J // 2, 4, 2], BF16,
                                        name=f"oh{half}")
                        tq = fld[tn]
                        tqs = tq[:].rearrange("p (jp jj) -> p jp jj", jj=2)
                        nc.vector.tensor_tensor(
                            out=oh[:],
                            in0=tqs.unsqueeze(2).broadcast_to([128, J // 2, 4, 2]),
                            in1=iota4[:].unsqueeze(1).unsqueeze(3)
                                .broadcast_to([128, J // 2, 4, 2]),
                            op=OP.is_equal)
                        # F1[jp, (half,q,k), jj] = Tu[jp, k, jj] * oh[jp, q, jj]
                        for jj in range(2):
                            nc.vector.tensor_tensor(
                                out=Fv[:, half, :, :, :, jj],
                                in0=Tu[:, :, :, jj].unsqueeze(2)
                                    .broadcast_to([128, J // 2, 4, 8]),
                                in1=oh[:, :, :, jj].unsqueeze(3)
                                    .broadcast_to([128, J // 2, 4, 8]),
                                op=OP.mult)

                    oh_ti = wpool.tile([128, J, 4], BF16)
                    nc.vector.tensor_tensor(
                        out=oh_ti[:],
                        in0=fld["ti"][:].unsqueeze(2).broadcast_to([128, J, 4]),
                        in1=iota4[:].unsqueeze(1).broadcast_to([128, J, 4]),
                        op=OP.is_equal)

                    # Legendre (bf16): P0=1, P1=ct, P2=1.5ct^2-.5, P3=ct(2.5ct^2-1.5)
                    ctf = fld["ct"]
                    P4 = wpool.tile([128, J, 4], BF16)
                    ct2 = wpool.tile([128, J], F32)
                    nc.vector.tensor_tensor(out=ct2[:], in0=ctf[:], in1=ctf[:],
                                            op=OP.mult)
                    nc.vector.memset(P4[:, :, 0], 1.0)
                    nc.vector.tensor_copy(out=P4[:, :, 1], in_=ctf[:])
                    nc.vector.tensor_scalar(out=P4[:, :, 2], in0=ct2[:], scalar1=1.5,
                                            scalar2=-0.5, op0=OP.mult, op1=OP.add)
                    p3t = wpool.tile([128, J], F32)
                    nc.vector.tensor_scalar(out=p3t[:], in0=ct2[:], scalar1=2.5,
                                            scalar2=-1.5, op0=OP.mult, op1=OP.add)
                    nc.vector.tensor_tensor(out=p3t[:], in0=p3t[:], in1=ctf[:],
                                            op=OP.mult)
                    nc.vector.tensor_copy(out=P4[:, :, 3], in_=p3t[:])

                    # ---- PE: per 8-column group; product once per 4 groups ----
                    prodc = prpool.tile([128, J, 32], BF16)
                    F1m = F1[:].rearrange("p jp f jj -> p jp (f jj)")
                    NG = J // 8
                    for g4 in range(NG // 4):
                        cndb = cndpool.tile([128, 4, 4, 128], BF16, name="cndb")
                        for gs in range(4):
                            grp = g4 * 4 + gs
                            ps1 = ppool1.tile([128, 4, 128], BF16, space="PSUM",
                                              name="ps1")
                            for c in range(4):
                                nc.tensor.transpose(
                                    out=ps1[:, c, :], in_=F1m[:, 4 * grp + c, :],
                                    identity=ident[:])
                            cin = cpool.tile([128, 4, 128], BF16, name="cin")
                            nc.scalar.activation(
                                cin[:].rearrange("p c f -> p (c f)"),
                                ps1[:].rearrange("p c f -> p (c f)"), AF.Identity)
                            ps2 = ppool2.tile([128, 4, 128], F32, space="PSUM",
                                              name="ps2")
                            for c in range(4):
                                nc.tensor.matmul(out=ps2[:, c, :], lhsT=cin[:, c, :],
                                                 rhs=w4[:], start=True, stop=True)
                            nc.scalar.activation(
                                cndb[:, gs, :, :].rearrange("p c f -> p (c f)"),
                                ps2[:].rearrange("p c f -> p (c f)"), AF.Identity)
                        # prodc[8(4g4+gs)+2c+jj, f] = g_ij * g_ik
                        psv = cndb[:].rearrange("p g c (j h f) -> p (g c) j h f",
                                                j=2, h=2)
                        nc.vector.tensor_tensor(
                            out=prodc[:, g4 * 32:(g4 + 1) * 32, :]
                                .rearrange("p (gc j) f -> p gc j f", j=2),
                            in0=psv[:, :, :, 0, :], in1=psv[:, :, :, 1, :],
                            op=OP.mult)

                    # ---- ti select: h = sum_q ohti[q] * prodc[:, :, q*8:(q+1)*8]
                    h = wpool.tile([128, J, 8], BF16)
                    t0 = wpool.tile([128, J, 8], BF16)
                    nc.vector.tensor_tensor(
                        out=h[:], in0=prodc[:, :, 0:8],
                        in1=oh_ti[:, :, 0].unsqueeze(2).broadcast_to([128, J, 8]),
                        op=OP.mult)
                    for q in range(1, 4):
                        nc.vector.tensor_tensor(
                            out=t0[:], in0=prodc[:, :, q * 8:(q + 1) * 8],
                            in1=oh_ti[:, :, q].unsqueeze(2).broadcast_to([128, J, 8]),
                            op=OP.mult)
                        nc.vector.tensor_tensor(out=h[:], in0=h[:], in1=t0[:],
                                                op=OP.add)

                    # ---- ang = h (x) P (bf16) ----
                    ang = apool.tile([128, J, 8, 4], BF16)
                    nc.vector.tensor_tensor(
                        out=ang[:],
                        in0=h[:].unsqueeze(3).broadcast_to([128, J, 8, 4]),
                        in1=P4[:].unsqueeze(2).broadcast_to([128, J, 8, 4]),
                        op=OP.mult)

                    # ---- per-block pre-reduction: 3 adds over 4 columns ----
                    partials = spool.tile([128, J // SEG, 32], F32)
                    av = ang[:].rearrange("p (g s) d l -> p g s (d l)", s=SEG)
                    ts1 = wpool.tile([128, J // SEG, 32], BF16)
                    ts2 = wpool.tile([128, J // SEG, 32], BF16)
                    nc.vector.tensor_tensor(out=ts1[:], in0=av[:, :, 0, :],
                                            in1=av[:, :, 1, :], op=OP.add)
                    nc.vector.tensor_tensor(out=ts2[:], in0=av[:, :, 2, :],
                                            in1=av[:, :, 3, :], op=OP.add)
                    nc.vector.tensor_tensor(out=partials[:], in0=ts1[:],
                                            in1=ts2[:], op=OP.add)

                    if debug and m == 0:
                        for nm, src, w in (
                            ("f1", F1[:].rearrange("p jp f jj -> p (jp f jj)"), 64 * J),
                            ("prodc", prodc[:].rearrange("p j f -> p (j f)"), J * 32),
                            ("partials", partials[:].rearrange("p g f -> p (g f)"),
                             32 * 32),
                        ):
                            for i in range(0, w, 1024):
                                e = min(i + 1024, w)
                                nc.vector.tensor_copy(out=dbg_t[:, :e - i],
                                                      in_=src[:, i:e])
                                nc.sync.dma_start(out=dbg[nm].ap()[:, i:e],
                                                  in_=dbg_t[:, :e - i])

                    # ---- scatter chunks of this macro (block granularity) ----
                    for (ci, off, pl) in by_macro[m]:
                        nc.gpsimd.dma_scatter_add(
                            qacc[ci % KACC].ap()[:, :DL],
                            partials[:, off // 128:(off + pl) // 128, :],
                            idxs[:, off // 16:(off + pl) // 16],
                            pl, pl, DL, elem_step=STRIDE,
                            queue_num=ci % 2)

            # ---- reduce K accumulators, AllReduce, emit ----
            with tc.tile_pool(name="red", bufs=1) as rpool:
                W = QPAD * STRIDE // 128   # 10240
                acc = rpool.tile([128, W], F32)
                tmp = rpool.tile([128, W], F32)
                qv = [q.ap().rearrange("(p r) s -> p (r s)", p=128) for q in qacc]
                nc.sync.dma_start(out=acc[:], in_=qv[0])
                nc.sync.dma_start(out=tmp[:], in_=qv[1])
                nc.vector.tensor_tensor(out=acc[:], in0=acc[:], in1=tmp[:], op=OP.add)
                packed = rpool.tile([128, QPAD * DL // 128], BF16)  # [128, 5120]
                nc.vector.tensor_copy(
                    out=packed[:].rearrange("p (r s) -> p r s", s=DL),
                    in_=acc[:].rearrange("p (r s) -> p r s", s=STRIDE)[:, :, :DL])
                nc.sync.dma_start(out=bounce_in.ap(), in_=packed[:])
                nc.gpsimd.collective_compute(
                    "AllReduce", OP.add,
                    replica_groups=[list(range(N_CORES))],
                    ins=[bounce_in.ap().opt()], outs=[bounce_out.ap().opt()])
                obf = rpool.tile([128, QPAD * DL // 128], BF16)
                nc.sync.dma_start(out=obf[:], in_=bounce_out.ap())
                of32 = rpool.tile([128, QPAD * DL // 128], F32)
                nc.vector.tensor_copy(out=of32[:], in_=obf[:])
                nc.sync.dma_start(
                    out=out_d.ap().rearrange("(p r) c -> p (r c)", r=QPAD // 128),
                    in_=of32[:N_ATOMS * 128 // QPAD, :])
    nc.compile()
    return nc


def _install_ntff_hook():
    """Provide antenv.axon_hooks (missing in this image) via sys.modules so
    run_bass_kernel_spmd(trace=True) can capture NTFF profiles."""
    import types, ctypes, contextlib
    try:
        from antenv.axon_hooks import get_axon_ntff_profile_hook  # noqa: F401
        return
    except ImportError:
        pass
    so_path = "/opt/axon/libaxon_pjrt.so"
    try:
        lib = ctypes.CDLL(so_path)
    except OSError:
        return
    if not hasattr(lib, "axon_start_nrt_profile"):
        return
    lib.axon_start_nrt_profile.argtypes = [ctypes.POINTER(ctypes.c_int64),
                                           ctypes.c_size_t]
    lib.axon_start_nrt_profile.restype = ctypes.c_int64
    lib.axon_stop_nrt_profile.argtypes = [ctypes.c_char_p]
    lib.axon_stop_nrt_profile.restype = ctypes.c_int64

    @contextlib.contextmanager
    def _hook(output_dir, device_ids):
        import jax
        jax.devices()
        if device_ids:
            ids = (ctypes.c_int64 * len(device_ids))(*device_ids)
            rc = lib.axon_start_nrt_profile(ids, len(device_ids))
        else:
            rc = lib.axon_start_nrt_profile(None, 0)
        if rc != 0:
            raise RuntimeError(f"axon_start_nrt_profile rc={rc}")
        try:
            yield
        finally:
            n = lib.axon_stop_nrt_profile(str(output_dir).encode())
            if n <= 0:
                print(f"ntff capture wrote {n} files", flush=True)

    mod = types.ModuleType("antenv.axon_hooks")
    mod.get_axon_ntff_profile_hook = lambda: _hook
    mod.set_axon_ntff_profile_hook = lambda h: None
    import antenv
    sys.modules["antenv.axon_hooks"] = mod
    antenv.axon_hooks = mod



_CACHE = {}


def kernel(n_atoms, triplet_index, r_ij, r_ik, cos_theta,
           type_i, type_j, type_k, c_table, _sim=False, _trace=False):
    cores, consts, chunks, nmacro, TPAD = _host_prep(
        n_atoms, triplet_index, r_ij, r_ik, cos_theta,
        type_i, type_j, type_k, c_table)
    key = (nmacro, TPAD, tuple(chunks))
    if key not in _CACHE:
        _CACHE[key] = _build(chunks, nmacro, TPAD)
    nc = _CACHE[key]
    in_maps = []
    for c in range(N_CORES):
        m = dict(cores[c])
        m.update(consts)
        in_maps.append(m)
    if _sim:
        from concourse import bass_interp
        sim = bass_interp.MultiCoreSim(nc, N_CORES)
        for c in range(N_CORES):
            for k, v in in_maps[c].items():
                sim.cores[c].tensor(k)[:] = v
        sim.simulate()
        out = np.array(sim.cores[0].mem_tensor("out"))
    else:
        if _trace:
            _install_ntff_hook()
        last_err = None
        for _try in range(3):
            try:
                res = run_bass_kernel_spmd(nc, in_maps,
                                           core_ids=list(range(N_CORES)),
                                           trace=_trace)
                out = np.asarray(res.results[0]["out"])
                break
            except Exception as e:  # transient device-unrecoverable after a crash
                last_err = e
        else:
            raise last_err
        kernel.last_exec_ns = res.exec_time_ns
        kernel.last_results = res
    return out.reshape(N_ATOMS, N_DESC, L_MAX).astype(np.float32)
